# revision 2
# baseline (speedup 1.0000x reference)
"""MultiHeadedAttention Trainium2 Bass kernel.

Full inputs -> full outputs. Shards batch (B=8) across 8 NeuronCores,
one batch element per core. Self-contained: hardcodes all shapes.

Math per core (batch item b):
  q = Wq @ query + bq  (channels o = d*4 + h permuted to head-blocked r = h*64 + d,
                        1/sqrt(64) folded into Wq/bq)
  k = Wk @ key + bk
  Vt[n, r] = (Wv @ value + bv)^T   (computed directly in transposed layout)
  per head h: S^T[m, n] = k_h^T-chunks x q_h ; E = exp(S^T) (no max subtraction:
              scores ~ N(0,1), exp is safe in fp32)
  x'[d, n]  = sum_m Vt_aug[m, d] * E[m, n]  with Vt_aug's 65th column = ones
              so row 64 of x' = softmax denominator Z[n]
  X[r, n]   = x'[d, n] / Z[n]
  out = Wm @ X + bm   (Wm columns pre-permuted to consume head-blocked X)
"""

import numpy as np

B = 8
D = 256
N = 2048
H = 4
HD = 64
NQ = 512            # unit column width (n-quarter)
NUNITS = H * (N // NQ)   # 16 units of (head, n-quarter)
NCHUNKS = 16        # m-chunks of 128 per unit
RING = 6            # psum score ring slots of [128, NQ]
ERING = 32          # E ring slots of [128, NQ] (2 units worth)

_CACHE = {}


def _build_nc():
    import concourse.bacc as bacc
    import concourse.mybir as mybir
    import concourse.tile as tile

    F32 = mybir.dt.float32
    F32R = mybir.dt.float32r
    BF16 = mybir.dt.bfloat16
    Exp = mybir.ActivationFunctionType.Exp

    nc = bacc.Bacc("TRN2", target_bir_lowering=False, debug=False, num_devices=B)

    # DRAM I/O (per-core shapes)
    d_q = nc.dram_tensor("query", [D, N], F32, kind="ExternalInput")
    d_k = nc.dram_tensor("key", [D, N], F32, kind="ExternalInput")
    d_v = nc.dram_tensor("value", [D, N], F32, kind="ExternalInput")
    d_wqt = nc.dram_tensor("wqt", [D, D], F32, kind="ExternalInput")
    d_wkt = nc.dram_tensor("wkt", [D, D], F32, kind="ExternalInput")
    d_wvt = nc.dram_tensor("wvt", [D, D], F32, kind="ExternalInput")
    d_wmt = nc.dram_tensor("wmt", [D, D], F32, kind="ExternalInput")
    d_bq = nc.dram_tensor("bq", [D, 1], F32, kind="ExternalInput")
    d_bk = nc.dram_tensor("bk", [D, 1], F32, kind="ExternalInput")
    d_bvb = nc.dram_tensor("bvb", [128, D], F32, kind="ExternalInput")
    d_bm = nc.dram_tensor("bm", [D, 1], F32, kind="ExternalInput")
    d_out = nc.dram_tensor("out", [D, N], F32, kind="ExternalOutput")

    with tile.TileContext(nc) as tc:
        with (
            tc.tile_pool(name="pers", bufs=1) as pers,
            tc.tile_pool(name="epool", bufs=1) as epool,
            tc.tile_pool(name="norm", bufs=3) as normp,
        ):
            # ---- persistent SBUF tiles ----
            qin = [pers.tile([128, N], F32R, tag=f"qin{i}", name=f"qin{i}") for i in range(2)]
            kin = [pers.tile([128, N], F32R, tag=f"kin{i}", name=f"kin{i}") for i in range(2)]
            vin = [pers.tile([128, N], F32R, tag=f"vin{i}", name=f"vin{i}") for i in range(2)]
            wqt = [pers.tile([128, D], F32R, tag=f"wqt{i}", name=f"wqt{i}") for i in range(2)]
            wkt = [pers.tile([128, D], F32R, tag=f"wkt{i}", name=f"wkt{i}") for i in range(2)]
            wvt = [pers.tile([128, D], F32R, tag=f"wvt{i}", name=f"wvt{i}") for i in range(2)]
            wmt = [pers.tile([128, D], F32R, tag=f"wmt{i}", name=f"wmt{i}") for i in range(2)]
            bq = [pers.tile([128, 1], F32, tag=f"bq{i}", name=f"bq{i}") for i in range(2)]
            bk = [pers.tile([128, 1], F32, tag=f"bk{i}", name=f"bk{i}") for i in range(2)]
            bm = [pers.tile([128, 1], F32, tag=f"bm{i}", name=f"bm{i}") for i in range(2)]
            bvb = pers.tile([128, D], F32, tag="bvb", name="bvb")
            q_sb = [pers.tile([128, N], F32R, tag=f"q{i}", name=f"q{i}") for i in range(2)]
            k_sb = [pers.tile([128, N], F32R, tag=f"k{i}", name=f"k{i}") for i in range(2)]
            x_sb = [pers.tile([128, N], F32R, tag=f"x{i}", name=f"x{i}") for i in range(2)]
            o_sb = [pers.tile([128, N], F32, tag=f"o{i}", name=f"o{i}") for i in range(2)]
            vt = pers.tile([128, NCHUNKS, H, HD + 1], BF16, tag="vt", name="vt")
            e_ring = epool.tile([128, ERING, NQ], BF16, tag="E", name="E")
            warm = pers.tile([1, 8], F32, tag="warm", name="warm")

            # ---- warm up the exp table on ACT as early as possible ----
            nc.vector.memset(warm[:], 0.0)
            nc.scalar.activation(out=warm[:], in_=warm[:], func=Exp)

            # ---- input DMAs ----
            for i in range(2):
                rows = slice(i * 128, (i + 1) * 128)
                nc.sync.dma_start(out=qin[i], in_=d_q[rows, :].bitcast(F32R))
                nc.sync.dma_start(out=kin[i], in_=d_k[rows, :].bitcast(F32R))
                nc.sync.dma_start(out=vin[i], in_=d_v[rows, :].bitcast(F32R))
                nc.sync.dma_start(out=wqt[i], in_=d_wqt[rows, :].bitcast(F32R))
                nc.sync.dma_start(out=wkt[i], in_=d_wkt[rows, :].bitcast(F32R))
                nc.sync.dma_start(out=wvt[i], in_=d_wvt[rows, :].bitcast(F32R))
                nc.sync.dma_start(out=wmt[i], in_=d_wmt[rows, :].bitcast(F32R))
                nc.sync.dma_start(out=bq[i], in_=d_bq[rows, :])
                nc.sync.dma_start(out=bk[i], in_=d_bk[rows, :])
                nc.sync.dma_start(out=bm[i], in_=d_bm[rows, :])
            nc.sync.dma_start(out=bvb, in_=d_bvb[:, :])

            # ones columns of vt (data columns overwritten below)
            nc.gpsimd.memset(vt[:, :, :, HD], 1.0)

            # ---- projections ----
            with tc.tile_pool(name="pp", bufs=2, space="PSUM") as pp:
                # Q and K: head-blocked [256, 2048], rows r = h*64+d.
                # mh0 first so heads 0/1 unblock the attention units early.
                def qk_round(dst, w, bias, mh, nh, src):
                    ps = pp.tile([128, 1024], F32, tag="pqk", name="pqk")
                    for ih in range(2):
                        for nt in range(2):
                            cols = slice(nh * 1024 + nt * NQ, nh * 1024 + (nt + 1) * NQ)
                            nc.tensor.matmul(
                                ps[:, nt * NQ:(nt + 1) * NQ],
                                w[ih][:, mh * 128:(mh + 1) * 128],
                                src[ih][:, cols],
                                start=(ih == 0), stop=(ih == 1),
                            )
                    cols = slice(nh * 1024, (nh + 1) * 1024)
                    nc.vector.tensor_scalar_add(
                        out=dst[mh][:, cols], in0=ps[:], scalar1=bias[mh])

                for nh in range(2):
                    qk_round(q_sb, wqt, bq, 0, nh, qin)
                for nh in range(2):
                    qk_round(k_sb, wkt, bk, 0, nh, kin)

                # Vt: [n, r] chunks with bias add + bf16 convert into vt layout
                for c in range(NCHUNKS):
                    pv = pp.tile([128, D], F32, tag="pvt", name="pvt")
                    for ih in range(2):
                        nc.tensor.matmul(
                            pv[:],
                            vin[ih][:, c * 128:(c + 1) * 128],
                            wvt[ih][:],
                            start=(ih == 0), stop=(ih == 1),
                        )
                    nc.vector.tensor_tensor(
                        out=vt[:, c, :, 0:HD],
                        in0=pv[:].rearrange("p (h d) -> p h d", h=H),
                        in1=bvb[:].rearrange("p (h d) -> p h d", h=H),
                        op=mybir.AluOpType.add,
                    )

                for nh in range(2):
                    qk_round(q_sb, wqt, bq, 1, nh, qin)
                for nh in range(2):
                    qk_round(k_sb, wkt, bk, 1, nh, kin)

            # ---- attention units ----
            with (
                tc.tile_pool(name="sring", bufs=1, space="PSUM") as srp,
                tc.tile_pool(name="xacc", bufs=2, space="PSUM") as xap,
            ):
                sring = srp.tile([128, RING, NQ], F32, tag="s", name="s")
                NG = NUNITS * NCHUNKS  # 256 global chunks

                def emit_S(g):
                    u, c = divmod(g, NCHUNKS)
                    h, qj = divmod(u, N // NQ)
                    th, hp = divmod(h, 2)
                    rows = slice(hp * 64, (hp + 1) * 64)
                    nc.tensor.matmul(
                        sring[:, g % RING, :],
                        k_sb[th][rows, c * 128:(c + 1) * 128],
                        q_sb[th][rows, qj * NQ:(qj + 1) * NQ],
                        start=True, stop=True, skip_group_check=True,
                    )

                def emit_exp(gs):  # gs: consecutive global chunks, same ring block
                    # split on E-ring wraparound
                    i = 0
                    while i < len(gs):
                        es = gs[i] % ERING
                        k = 1
                        while (i + k < len(gs)) and (es + k < ERING):
                            k += 1
                        s0 = gs[i] % RING
                        nc.scalar.activation(
                            out=e_ring[:, es:es + k, :],
                            in_=sring[:, s0:s0 + k, :],
                            func=Exp,
                        )
                        i += k

                xaccs = {}

                def emit_PV(g):
                    u, c = divmod(g, NCHUNKS)
                    h = u // (N // NQ)
                    if c == 0:
                        xaccs[u] = xap.tile([HD + 1, NQ], F32, tag="xa", name="xa")
                    nc.tensor.matmul(
                        xaccs[u][:],
                        vt[:, c, h, :],
                        e_ring[:, g % ERING, :],
                        start=(c == 0), stop=(c == NCHUNKS - 1),
                        skip_group_check=True,
                    )

                def emit_norm(u):
                    h, qj = divmod(u, N // NQ)
                    th, hp = divmod(h, 2)
                    xa = xaccs.pop(u)
                    zrow = normp.tile([1, NQ], F32, tag="zrow", name="zrow")
                    nc.vector.tensor_copy(out=zrow[:], in_=xa[HD:HD + 1, :])
                    zrec = normp.tile([1, NQ], F32, tag="zrec", name="zrec")
                    nc.vector.reciprocal_approx_fast(out=zrec[:], in_=zrow[:])
                    zb = normp.tile([64, NQ], F32, tag="zb", name="zb")
                    nc.gpsimd.partition_broadcast(zb[:], zrec[:])
                    nc.vector.tensor_tensor(
                        out=x_sb[th][hp * 64:(hp + 1) * 64, qj * NQ:(qj + 1) * NQ],
                        in0=xa[0:HD, :],
                        in1=zb[:],
                        op=mybir.AluOpType.mult,
                    )

                # triad-pipelined emission
                triads = [list(range(t * 3, min(t * 3 + 3, NG)))
                          for t in range((NG + 2) // 3)]
                for t, chunks in enumerate(triads):
                    for g in chunks:
                        emit_S(g)
                    if t >= 1:
                        emit_exp(triads[t - 1])
                        for g in triads[t - 1]:
                            emit_PV(g)
                            if g % NCHUNKS == NCHUNKS - 1:
                                emit_norm(g // NCHUNKS)
                emit_exp(triads[-1])
                for g in triads[-1]:
                    emit_PV(g)
                    if g % NCHUNKS == NCHUNKS - 1:
                        emit_norm(g // NCHUNKS)

            # ---- output projection ----
            with tc.tile_pool(name="po", bufs=2, space="PSUM") as pop:
                for mh in range(2):
                    for nh in range(2):
                        ps = pop.tile([128, 1024], F32, tag="po", name="po")
                        for ih in range(2):
                            for nt in range(2):
                                cols = slice(nh * 1024 + nt * NQ,
                                             nh * 1024 + (nt + 1) * NQ)
                                nc.tensor.matmul(
                                    ps[:, nt * NQ:(nt + 1) * NQ],
                                    wmt[ih][:, mh * 128:(mh + 1) * 128],
                                    x_sb[ih][:, cols],
                                    start=(ih == 0), stop=(ih == 1),
                                )
                        cols = slice(nh * 1024, (nh + 1) * 1024)
                        nc.vector.tensor_scalar_add(
                            out=o_sb[mh][:, cols], in0=ps[:], scalar1=bm[mh])
                    nc.sync.dma_start(
                        out=d_out[mh * 128:(mh + 1) * 128, :], in_=o_sb[mh][:])

    nc.finalize()
    return nc


def _get_nc():
    if "nc" not in _CACHE:
        _CACHE["nc"] = _build_nc()
    return _CACHE["nc"]


def _prep_host(Wq, bq, Wk, bk, Wv, bv, Wm, bm):
    r = np.arange(D)
    perm = (r % HD) * H + (r // HD)  # head-blocked row r -> original channel o
    s = np.float32(1.0 / np.sqrt(HD))
    f32 = np.float32
    wqt = np.ascontiguousarray((Wq[perm, :] * s).T, dtype=f32)
    bq_p = np.ascontiguousarray((bq[perm] * s)[:, None], dtype=f32)
    wkt = np.ascontiguousarray(Wk[perm, :].T, dtype=f32)
    bk_p = np.ascontiguousarray(bk[perm][:, None], dtype=f32)
    wvt = np.ascontiguousarray(Wv[perm, :].T, dtype=f32)
    bvb = np.ascontiguousarray(np.tile(bv[perm][None, :], (128, 1)), dtype=f32)
    wmt = np.ascontiguousarray(Wm[:, perm].T, dtype=f32)
    bm_p = np.ascontiguousarray(bm[:, None], dtype=f32)
    return dict(wqt=wqt, bq=bq_p, wkt=wkt, bk=bk_p, wvt=wvt, bvb=bvb,
                wmt=wmt, bm=bm_p)


def _run(inputs, trace=False):
    from concourse.bass_utils import run_bass_kernel_spmd

    query = np.ascontiguousarray(np.asarray(inputs["query"], dtype=np.float32))
    key = np.ascontiguousarray(np.asarray(inputs["key"], dtype=np.float32))
    value = np.ascontiguousarray(np.asarray(inputs["value"], dtype=np.float32))
    w = _prep_host(
        np.asarray(inputs["Wq"], np.float32), np.asarray(inputs["bq"], np.float32),
        np.asarray(inputs["Wk"], np.float32), np.asarray(inputs["bk"], np.float32),
        np.asarray(inputs["Wv"], np.float32), np.asarray(inputs["bv"], np.float32),
        np.asarray(inputs["Wm"], np.float32), np.asarray(inputs["bm"], np.float32),
    )
    in_maps = []
    for b in range(B):
        m = dict(w)
        m["query"] = np.ascontiguousarray(query[b])
        m["key"] = np.ascontiguousarray(key[b])
        m["value"] = np.ascontiguousarray(value[b])
        in_maps.append(m)
    nc = _get_nc()
    res = run_bass_kernel_spmd(nc, in_maps, core_ids=list(range(B)), trace=trace)
    out = np.stack([r["out"] for r in res.results], axis=0)
    return out, res


def kernel(**inputs):
    out, _ = _run(inputs, trace=False)
    return out


if __name__ == "__main__":
    rng = np.random.default_rng(0)
    s = 1.0 / np.sqrt(D)
    inputs = {
        "query": rng.standard_normal((B, D, N), dtype=np.float32),
        "key": rng.standard_normal((B, D, N), dtype=np.float32),
        "value": rng.standard_normal((B, D, N), dtype=np.float32),
        "Wq": rng.standard_normal((D, D), dtype=np.float32) * s,
        "bq": rng.standard_normal((D,), dtype=np.float32) * 0.01,
        "Wk": rng.standard_normal((D, D), dtype=np.float32) * s,
        "bk": rng.standard_normal((D,), dtype=np.float32) * 0.01,
        "Wv": rng.standard_normal((D, D), dtype=np.float32) * s,
        "bv": rng.standard_normal((D,), dtype=np.float32) * 0.01,
        "Wm": rng.standard_normal((D, D), dtype=np.float32) * s,
        "bm": rng.standard_normal((D,), dtype=np.float32) * 0.01,
    }
    out = kernel(**inputs)
    # numpy reference
    def proj(x, W, b):
        return np.einsum("oi,bin->bon", W, x) + b[None, :, None]
    q = proj(inputs["query"], inputs["Wq"], inputs["bq"]).reshape(B, HD, H, N)
    k = proj(inputs["key"], inputs["Wk"], inputs["bk"]).reshape(B, HD, H, N)
    v = proj(inputs["value"], inputs["Wv"], inputs["bv"]).reshape(B, HD, H, N)
    sc = np.einsum("bdhn,bdhm->bhnm", q, k) / np.sqrt(HD)
    sc = sc - sc.max(axis=-1, keepdims=True)
    p = np.exp(sc)
    p /= p.sum(axis=-1, keepdims=True)
    x = np.einsum("bhnm,bdhm->bdhn", p, v).reshape(B, D, N)
    ref = proj(x, inputs["Wm"], inputs["bm"])
    err = np.abs(out - ref)
    scale = np.abs(ref).max()
    print("abs err max:", err.max(), "scaled:", err.max() / scale)
    rel = np.linalg.norm(out - ref) / np.linalg.norm(ref)
    print("fro rel err:", rel)


# revision 3
# speedup vs baseline: 1.3405x; 1.3405x over previous
"""MultiHeadedAttention Trainium2 Bass kernel.

Full inputs -> full outputs. Shards batch (B=8) across 8 NeuronCores,
one batch element per core. Self-contained: hardcodes all shapes.

Math per core (batch item b):
  q = Wq @ query + bq  (channels o = d*4 + h permuted to head-blocked r = h*64 + d,
                        1/sqrt(64) folded into Wq/bq)
  k = Wk @ key + bk
  Vt[n, r] = (Wv @ value + bv)^T   (computed directly in transposed layout)
  per head h: S^T[m, n] = k_h^T-chunks x q_h ; E = exp(S^T) (no max subtraction:
              scores ~ N(0,1), exp is safe in fp32)
  x'[d, n]  = sum_m Vt_aug[m, d] * E[m, n]  with Vt_aug's 65th column = ones
              so row 64 of x' = softmax denominator Z[n]
  X[r, n]   = x'[d, n] / Z[n]
  out = Wm @ X + bm   (Wm columns pre-permuted to consume head-blocked X)
"""

import numpy as np

B = 8
D = 256
N = 2048
H = 4
HD = 64
NQ = 512            # unit column width (n-quarter)
NUNITS = H * (N // NQ)   # 16 units of (head, n-quarter)
NCHUNKS = 16        # m-chunks of 128 per unit
RING = 6            # psum score ring slots of [128, NQ]
ERING = 32          # E ring slots of [128, NQ] (2 units worth)

_CACHE = {}


def _build_nc():
    import concourse.bacc as bacc
    import concourse.mybir as mybir
    import concourse.tile as tile

    F32 = mybir.dt.float32
    F32R = mybir.dt.float32r
    BF16 = mybir.dt.bfloat16
    Exp = mybir.ActivationFunctionType.Exp

    nc = bacc.Bacc("TRN2", target_bir_lowering=False, debug=False, num_devices=B)

    # DRAM I/O (per-core shapes)
    d_q = nc.dram_tensor("query", [D, N], F32, kind="ExternalInput")
    d_k = nc.dram_tensor("key", [D, N], F32, kind="ExternalInput")
    d_v = nc.dram_tensor("value", [D, N], F32, kind="ExternalInput")
    d_wqt = nc.dram_tensor("wqt", [D, D], F32, kind="ExternalInput")
    d_wkt = nc.dram_tensor("wkt", [D, D], F32, kind="ExternalInput")
    d_wvt = nc.dram_tensor("wvt", [D, D], F32, kind="ExternalInput")
    d_wmt = nc.dram_tensor("wmt", [D, D], F32, kind="ExternalInput")
    d_bq = nc.dram_tensor("bq", [D, 1], F32, kind="ExternalInput")
    d_bk = nc.dram_tensor("bk", [D, 1], F32, kind="ExternalInput")
    d_bvb = nc.dram_tensor("bvb", [128, D], F32, kind="ExternalInput")
    d_bm = nc.dram_tensor("bm", [D, 1], F32, kind="ExternalInput")
    d_out = nc.dram_tensor("out", [D, N], F32, kind="ExternalOutput")

    with tile.TileContext(nc) as tc:
        with (
            tc.tile_pool(name="pers", bufs=1) as pers,
            tc.tile_pool(name="epool", bufs=1) as epool,
            tc.tile_pool(name="norm", bufs=3) as normp,
        ):
            # ---- persistent SBUF tiles ----
            qin = [pers.tile([128, N], F32R, tag=f"qin{i}", name=f"qin{i}") for i in range(2)]
            kin = [pers.tile([128, N], F32R, tag=f"kin{i}", name=f"kin{i}") for i in range(2)]
            vin = [pers.tile([128, N], F32R, tag=f"vin{i}", name=f"vin{i}") for i in range(2)]
            wqt = [pers.tile([128, D], F32R, tag=f"wqt{i}", name=f"wqt{i}") for i in range(2)]
            wkt = [pers.tile([128, D], F32R, tag=f"wkt{i}", name=f"wkt{i}") for i in range(2)]
            wvt = [pers.tile([128, D], F32R, tag=f"wvt{i}", name=f"wvt{i}") for i in range(2)]
            wmt = [pers.tile([128, D], F32R, tag=f"wmt{i}", name=f"wmt{i}") for i in range(2)]
            bq = [pers.tile([128, 1], F32, tag=f"bq{i}", name=f"bq{i}") for i in range(2)]
            bk = [pers.tile([128, 1], F32, tag=f"bk{i}", name=f"bk{i}") for i in range(2)]
            bm = [pers.tile([128, 1], F32, tag=f"bm{i}", name=f"bm{i}") for i in range(2)]
            bvb = pers.tile([128, D], F32, tag="bvb", name="bvb")
            q_sb = [pers.tile([128, N], F32R, tag=f"q{i}", name=f"q{i}") for i in range(2)]
            k_sb = [pers.tile([128, N], F32R, tag=f"k{i}", name=f"k{i}") for i in range(2)]
            x_sb = [pers.tile([128, N], F32R, tag=f"x{i}", name=f"x{i}") for i in range(2)]
            o_sb = [pers.tile([128, N], F32, tag=f"o{i}", name=f"o{i}") for i in range(2)]
            vt = pers.tile([128, NCHUNKS, H, HD + 1], BF16, tag="vt", name="vt")
            e_ring = epool.tile([128, ERING, NQ], BF16, tag="E", name="E")
            warm = pers.tile([1, 8], F32, tag="warm", name="warm")

            # ---- warm up the exp table on ACT as early as possible ----
            nc.vector.memset(warm[:], 0.0)
            nc.scalar.activation(out=warm[:], in_=warm[:], func=Exp)

            # ---- input DMAs ----
            for i in range(2):
                rows = slice(i * 128, (i + 1) * 128)
                nc.sync.dma_start(out=qin[i], in_=d_q[rows, :].bitcast(F32R))
                nc.sync.dma_start(out=kin[i], in_=d_k[rows, :].bitcast(F32R))
                nc.sync.dma_start(out=vin[i], in_=d_v[rows, :].bitcast(F32R))
                nc.sync.dma_start(out=wqt[i], in_=d_wqt[rows, :].bitcast(F32R))
                nc.sync.dma_start(out=wkt[i], in_=d_wkt[rows, :].bitcast(F32R))
                nc.sync.dma_start(out=wvt[i], in_=d_wvt[rows, :].bitcast(F32R))
                nc.sync.dma_start(out=wmt[i], in_=d_wmt[rows, :].bitcast(F32R))
                nc.sync.dma_start(out=bq[i], in_=d_bq[rows, :])
                nc.sync.dma_start(out=bk[i], in_=d_bk[rows, :])
                nc.sync.dma_start(out=bm[i], in_=d_bm[rows, :])
            nc.sync.dma_start(out=bvb, in_=d_bvb[:, :])

            # ones columns of vt (data columns overwritten below)
            nc.gpsimd.memset(vt[:, :, :, HD], 1.0)

            # ---- projections ----
            with tc.tile_pool(name="pp", bufs=2, space="PSUM") as pp:
                # Q and K: head-blocked [256, 2048], rows r = h*64+d.
                # mh0 first so heads 0/1 unblock the attention units early.
                def qk_round(dst, w, bias, mh, nh, src):
                    ps = pp.tile([128, 1024], F32, tag="pqk", name="pqk")
                    for ih in range(2):
                        for nt in range(2):
                            cols = slice(nh * 1024 + nt * NQ, nh * 1024 + (nt + 1) * NQ)
                            nc.tensor.matmul(
                                ps[:, nt * NQ:(nt + 1) * NQ],
                                w[ih][:, mh * 128:(mh + 1) * 128],
                                src[ih][:, cols],
                                start=(ih == 0), stop=(ih == 1),
                            )
                    cols = slice(nh * 1024, (nh + 1) * 1024)
                    nc.vector.tensor_scalar_add(
                        out=dst[mh][:, cols], in0=ps[:], scalar1=bias[mh])

                for nh in range(2):
                    qk_round(q_sb, wqt, bq, 0, nh, qin)
                for nh in range(2):
                    qk_round(k_sb, wkt, bk, 0, nh, kin)

                # Vt: [n, r] chunks with bias add + bf16 convert into vt layout
                for c in range(NCHUNKS):
                    pv = pp.tile([128, D], F32, tag="pvt", name="pvt")
                    for ih in range(2):
                        nc.tensor.matmul(
                            pv[:],
                            vin[ih][:, c * 128:(c + 1) * 128],
                            wvt[ih][:],
                            start=(ih == 0), stop=(ih == 1),
                        )
                    nc.vector.tensor_tensor(
                        out=vt[:, c, :, 0:HD],
                        in0=pv[:].rearrange("p (h d) -> p h d", h=H),
                        in1=bvb[:].rearrange("p (h d) -> p h d", h=H),
                        op=mybir.AluOpType.add,
                    )

                for nh in range(2):
                    qk_round(q_sb, wqt, bq, 1, nh, qin)
                for nh in range(2):
                    qk_round(k_sb, wkt, bk, 1, nh, kin)

            # ---- attention units ----
            with (
                tc.tile_pool(name="sring", bufs=1, space="PSUM") as srp,
                tc.tile_pool(name="xacc", bufs=2, space="PSUM") as xap,
            ):
                sring = srp.tile([128, RING, NQ], F32, tag="s", name="s")
                NG = NUNITS * NCHUNKS  # 256 global chunks

                def emit_S(g):
                    u, c = divmod(g, NCHUNKS)
                    h, qj = divmod(u, N // NQ)
                    th, hp = divmod(h, 2)
                    rows = slice(hp * 64, (hp + 1) * 64)
                    nc.tensor.matmul(
                        sring[:, g % RING, :],
                        k_sb[th][rows, c * 128:(c + 1) * 128],
                        q_sb[th][rows, qj * NQ:(qj + 1) * NQ],
                        start=True, stop=True, skip_group_check=True,
                    )

                def emit_exp(gs):  # gs: consecutive global chunks, same ring block
                    # split on E-ring wraparound
                    i = 0
                    while i < len(gs):
                        es = gs[i] % ERING
                        k = 1
                        while (i + k < len(gs)) and (es + k < ERING):
                            k += 1
                        s0 = gs[i] % RING
                        nc.scalar.activation(
                            out=e_ring[:, es:es + k, :],
                            in_=sring[:, s0:s0 + k, :],
                            func=Exp,
                        )
                        i += k

                xaccs = {}

                def emit_PV(g):
                    u, c = divmod(g, NCHUNKS)
                    h = u // (N // NQ)
                    if c == 0:
                        xaccs[u] = xap.tile([HD + 1, NQ], F32, tag="xa", name="xa")
                    nc.tensor.matmul(
                        xaccs[u][:],
                        vt[:, c, h, :],
                        e_ring[:, g % ERING, :],
                        start=(c == 0), stop=(c == NCHUNKS - 1),
                        skip_group_check=True,
                    )

                def emit_norm(u):
                    h, qj = divmod(u, N // NQ)
                    th, hp = divmod(h, 2)
                    xa = xaccs.pop(u)
                    zrow = normp.tile([1, NQ], F32, tag="zrow", name="zrow")
                    nc.vector.tensor_copy(out=zrow[:], in_=xa[HD:HD + 1, :])
                    zrec = normp.tile([1, NQ], F32, tag="zrec", name="zrec")
                    nc.vector.reciprocal_approx_fast(out=zrec[:], in_=zrow[:])
                    zb = normp.tile([64, NQ], F32, tag="zb", name="zb")
                    nc.gpsimd.partition_broadcast(zb[:], zrec[:])
                    nc.vector.tensor_tensor(
                        out=x_sb[th][hp * 64:(hp + 1) * 64, qj * NQ:(qj + 1) * NQ],
                        in0=xa[0:HD, :],
                        in1=zb[:],
                        op=mybir.AluOpType.mult,
                    )

                # triad-pipelined emission: exp(T) directly after S(T) so no
                # later sring writer precedes it; PV(T-1) trails one triad.
                triads = [list(range(t * 3, min(t * 3 + 3, NG)))
                          for t in range((NG + 2) // 3)]
                for t, chunks in enumerate(triads):
                    for g in chunks:
                        emit_S(g)
                    emit_exp(chunks)
                    if t >= 1:
                        for g in triads[t - 1]:
                            emit_PV(g)
                            if g % NCHUNKS == NCHUNKS - 1:
                                emit_norm(g // NCHUNKS)
                for g in triads[-1]:
                    emit_PV(g)
                    if g % NCHUNKS == NCHUNKS - 1:
                        emit_norm(g // NCHUNKS)

            # ---- output projection ----
            with tc.tile_pool(name="po", bufs=2, space="PSUM") as pop:
                for mh in range(2):
                    for nh in range(2):
                        ps = pop.tile([128, 1024], F32, tag="po", name="po")
                        for ih in range(2):
                            for nt in range(2):
                                cols = slice(nh * 1024 + nt * NQ,
                                             nh * 1024 + (nt + 1) * NQ)
                                nc.tensor.matmul(
                                    ps[:, nt * NQ:(nt + 1) * NQ],
                                    wmt[ih][:, mh * 128:(mh + 1) * 128],
                                    x_sb[ih][:, cols],
                                    start=(ih == 0), stop=(ih == 1),
                                )
                        cols = slice(nh * 1024, (nh + 1) * 1024)
                        nc.vector.tensor_scalar_add(
                            out=o_sb[mh][:, cols], in0=ps[:], scalar1=bm[mh])
                    nc.sync.dma_start(
                        out=d_out[mh * 128:(mh + 1) * 128, :], in_=o_sb[mh][:])

    nc.finalize()
    return nc


def _get_nc():
    if "nc" not in _CACHE:
        _CACHE["nc"] = _build_nc()
    return _CACHE["nc"]


def _prep_host(Wq, bq, Wk, bk, Wv, bv, Wm, bm):
    r = np.arange(D)
    perm = (r % HD) * H + (r // HD)  # head-blocked row r -> original channel o
    s = np.float32(1.0 / np.sqrt(HD))
    f32 = np.float32
    wqt = np.ascontiguousarray((Wq[perm, :] * s).T, dtype=f32)
    bq_p = np.ascontiguousarray((bq[perm] * s)[:, None], dtype=f32)
    wkt = np.ascontiguousarray(Wk[perm, :].T, dtype=f32)
    bk_p = np.ascontiguousarray(bk[perm][:, None], dtype=f32)
    wvt = np.ascontiguousarray(Wv[perm, :].T, dtype=f32)
    bvb = np.ascontiguousarray(np.tile(bv[perm][None, :], (128, 1)), dtype=f32)
    wmt = np.ascontiguousarray(Wm[:, perm].T, dtype=f32)
    bm_p = np.ascontiguousarray(bm[:, None], dtype=f32)
    return dict(wqt=wqt, bq=bq_p, wkt=wkt, bk=bk_p, wvt=wvt, bvb=bvb,
                wmt=wmt, bm=bm_p)


def _run(inputs, trace=False):
    from concourse.bass_utils import run_bass_kernel_spmd

    query = np.ascontiguousarray(np.asarray(inputs["query"], dtype=np.float32))
    key = np.ascontiguousarray(np.asarray(inputs["key"], dtype=np.float32))
    value = np.ascontiguousarray(np.asarray(inputs["value"], dtype=np.float32))
    w = _prep_host(
        np.asarray(inputs["Wq"], np.float32), np.asarray(inputs["bq"], np.float32),
        np.asarray(inputs["Wk"], np.float32), np.asarray(inputs["bk"], np.float32),
        np.asarray(inputs["Wv"], np.float32), np.asarray(inputs["bv"], np.float32),
        np.asarray(inputs["Wm"], np.float32), np.asarray(inputs["bm"], np.float32),
    )
    in_maps = []
    for b in range(B):
        m = dict(w)
        m["query"] = np.ascontiguousarray(query[b])
        m["key"] = np.ascontiguousarray(key[b])
        m["value"] = np.ascontiguousarray(value[b])
        in_maps.append(m)
    nc = _get_nc()
    res = run_bass_kernel_spmd(nc, in_maps, core_ids=list(range(B)), trace=trace)
    out = np.stack([r["out"] for r in res.results], axis=0)
    return out, res


def kernel(**inputs):
    out, _ = _run(inputs, trace=False)
    return out


if __name__ == "__main__":
    rng = np.random.default_rng(0)
    s = 1.0 / np.sqrt(D)
    inputs = {
        "query": rng.standard_normal((B, D, N), dtype=np.float32),
        "key": rng.standard_normal((B, D, N), dtype=np.float32),
        "value": rng.standard_normal((B, D, N), dtype=np.float32),
        "Wq": rng.standard_normal((D, D), dtype=np.float32) * s,
        "bq": rng.standard_normal((D,), dtype=np.float32) * 0.01,
        "Wk": rng.standard_normal((D, D), dtype=np.float32) * s,
        "bk": rng.standard_normal((D,), dtype=np.float32) * 0.01,
        "Wv": rng.standard_normal((D, D), dtype=np.float32) * s,
        "bv": rng.standard_normal((D,), dtype=np.float32) * 0.01,
        "Wm": rng.standard_normal((D, D), dtype=np.float32) * s,
        "bm": rng.standard_normal((D,), dtype=np.float32) * 0.01,
    }
    out = kernel(**inputs)
    # numpy reference
    def proj(x, W, b):
        return np.einsum("oi,bin->bon", W, x) + b[None, :, None]
    q = proj(inputs["query"], inputs["Wq"], inputs["bq"]).reshape(B, HD, H, N)
    k = proj(inputs["key"], inputs["Wk"], inputs["bk"]).reshape(B, HD, H, N)
    v = proj(inputs["value"], inputs["Wv"], inputs["bv"]).reshape(B, HD, H, N)
    sc = np.einsum("bdhn,bdhm->bhnm", q, k) / np.sqrt(HD)
    sc = sc - sc.max(axis=-1, keepdims=True)
    p = np.exp(sc)
    p /= p.sum(axis=-1, keepdims=True)
    x = np.einsum("bhnm,bdhm->bdhn", p, v).reshape(B, D, N)
    ref = proj(x, inputs["Wm"], inputs["bm"])
    err = np.abs(out - ref)
    scale = np.abs(ref).max()
    print("abs err max:", err.max(), "scaled:", err.max() / scale)
    rel = np.linalg.norm(out - ref) / np.linalg.norm(ref)
    print("fro rel err:", rel)


# revision 8
# speedup vs baseline: 1.5100x; 1.1265x over previous
"""MultiHeadedAttention Trainium2 Bass kernel.

Full inputs -> full outputs. Shards batch (B=8) across 8 NeuronCores,
one batch element per core. Self-contained: hardcodes all shapes.

Math per core (batch item b):
  q = Wq @ query + bq  (channels o = d*4 + h permuted to head-blocked r = h*64 + d,
                        1/sqrt(64) folded into Wq/bq)
  k = Wk @ key + bk
  Vt[n, r] = (Wv @ value + bv)^T   (computed directly in transposed layout)
  per head h: S^T[m, n] = k_h^T-chunks x q_h ; E = exp(S^T) (no max subtraction:
              scores ~ N(0,1), exp is safe in fp32)
  x'[d, n]  = sum_m Vt_aug[m, d] * E[m, n]  with Vt_aug's 65th column = ones
              so row 64 of x' = softmax denominator Z[n]
  X[r, n]   = x'[d, n] / Z[n]
  out = Wm @ X + bm   (Wm columns pre-permuted to consume head-blocked X)
"""

import numpy as np

B = 8
D = 256
N = 2048
H = 4
HD = 64
NQ = 512            # unit column width (n-quarter)
NUNITS = H * (N // NQ)   # 16 units of (head, n-quarter)
NCHUNKS = 16        # m-chunks of 128 per unit
RING = 6            # psum score ring slots of [128, NQ]
ERING = 32          # E ring slots of [128, NQ] (2 units worth)

_CACHE = {}


def _build_nc():
    import concourse.bacc as bacc
    import concourse.mybir as mybir
    import concourse.tile as tile

    F32 = mybir.dt.float32
    F32R = mybir.dt.float32r
    BF16 = mybir.dt.bfloat16
    Exp = mybir.ActivationFunctionType.Exp

    nc = bacc.Bacc("TRN2", target_bir_lowering=False, debug=False, num_devices=B)

    # DRAM I/O (per-core shapes)
    d_q = nc.dram_tensor("query", [D, N], F32, kind="ExternalInput")
    d_k = nc.dram_tensor("key", [D, N], F32, kind="ExternalInput")
    d_v = nc.dram_tensor("value", [D, N], F32, kind="ExternalInput")
    d_wqt = nc.dram_tensor("wqt", [D, D], F32, kind="ExternalInput")
    d_wkt = nc.dram_tensor("wkt", [D, D], F32, kind="ExternalInput")
    d_wvt = nc.dram_tensor("wvt", [D, D], F32, kind="ExternalInput")
    d_wmt = nc.dram_tensor("wmt", [D, D], F32, kind="ExternalInput")
    d_bq = nc.dram_tensor("bq", [D, 1], F32, kind="ExternalInput")
    d_bk = nc.dram_tensor("bk", [D, 1], F32, kind="ExternalInput")
    d_bvb = nc.dram_tensor("bvb", [128, D], F32, kind="ExternalInput")
    d_bm = nc.dram_tensor("bm", [D, 1], F32, kind="ExternalInput")
    d_out = nc.dram_tensor("out", [D, N], F32, kind="ExternalOutput")

    with tile.TileContext(nc) as tc:
        with (
            tc.tile_pool(name="pers", bufs=1) as pers,
            tc.tile_pool(name="epool", bufs=1) as epool,
            tc.tile_pool(name="norm", bufs=3) as normp,
        ):
            # ---- persistent SBUF tiles ----
            qin = [pers.tile([128, N], F32R, tag=f"qin{i}", name=f"qin{i}") for i in range(2)]
            kin = [pers.tile([128, N], F32R, tag=f"kin{i}", name=f"kin{i}") for i in range(2)]
            vin = [pers.tile([128, N], F32R, tag=f"vin{i}", name=f"vin{i}") for i in range(2)]
            wqt = [pers.tile([128, D], F32R, tag=f"wqt{i}", name=f"wqt{i}") for i in range(2)]
            wkt = [pers.tile([128, D], F32R, tag=f"wkt{i}", name=f"wkt{i}") for i in range(2)]
            wvt = [pers.tile([128, D], F32R, tag=f"wvt{i}", name=f"wvt{i}") for i in range(2)]
            wmt = [pers.tile([128, D], F32R, tag=f"wmt{i}", name=f"wmt{i}") for i in range(2)]
            bq = [pers.tile([128, 1], F32, tag=f"bq{i}", name=f"bq{i}") for i in range(2)]
            bk = [pers.tile([128, 1], F32, tag=f"bk{i}", name=f"bk{i}") for i in range(2)]
            bm = [pers.tile([128, 1], F32, tag=f"bm{i}", name=f"bm{i}") for i in range(2)]
            bvb = pers.tile([128, D], F32, tag="bvb", name="bvb")
            q_sb = [pers.tile([128, N], F32R, tag=f"q{i}", name=f"q{i}") for i in range(2)]
            # K stored per head in zero-padded full-height tiles so the score
            # matmuls have K=128 (K=64 matmuls never HAM-warm and run ~2.4x slow)
            k_sb = [pers.tile([128, N], F32R, tag=f"k{i}", name=f"k{i}") for i in range(4)]
            x_sb = [pers.tile([128, N], F32R, tag=f"x{i}", name=f"x{i}") for i in range(2)]
            o_sb = [pers.tile([128, N], F32, tag=f"o{i}", name=f"o{i}") for i in range(2)]
            vt = pers.tile([128, NCHUNKS, H, HD + 1], BF16, tag="vt", name="vt")
            e_ring = epool.tile([128, ERING, NQ], BF16, tag="E", name="E")
            warm = pers.tile([1, 8], F32, tag="warm", name="warm")

            # ---- warm up the exp table on ACT as early as possible ----
            nc.vector.memset(warm[:], 0.0)
            nc.scalar.activation(out=warm[:], in_=warm[:], func=Exp)

            # ---- input DMAs ----
            for i in range(2):
                rows = slice(i * 128, (i + 1) * 128)
                nc.sync.dma_start(out=qin[i], in_=d_q[rows, :].bitcast(F32R))
                nc.sync.dma_start(out=kin[i], in_=d_k[rows, :].bitcast(F32R))
                nc.sync.dma_start(out=vin[i], in_=d_v[rows, :].bitcast(F32R))
                nc.sync.dma_start(out=wqt[i], in_=d_wqt[rows, :].bitcast(F32R))
                nc.sync.dma_start(out=wkt[i], in_=d_wkt[rows, :].bitcast(F32R))
                nc.sync.dma_start(out=wvt[i], in_=d_wvt[rows, :].bitcast(F32R))
                nc.sync.dma_start(out=wmt[i], in_=d_wmt[rows, :].bitcast(F32R))
                nc.sync.dma_start(out=bq[i], in_=d_bq[rows, :])
                nc.sync.dma_start(out=bk[i], in_=d_bk[rows, :])
                nc.sync.dma_start(out=bm[i], in_=d_bm[rows, :])
            nc.sync.dma_start(out=bvb, in_=d_bvb[:, :])

            # ones columns of vt (data columns overwritten below)
            nc.gpsimd.memset(vt[:, :, :, HD], 1.0)

            # ---- projections ----
            with tc.tile_pool(name="pp", bufs=2, space="PSUM") as pp:
                # Q and K: head-blocked [256, 2048], rows r = h*64+d.
                # mh0 first so heads 0/1 unblock the attention units early.
                def proj_round(w, mh, nh, src):
                    ps = pp.tile([128, 1024], F32, tag="pqk", name="pqk")
                    for ih in range(2):
                        for nt in range(2):
                            cols = slice(nh * 1024 + nt * NQ, nh * 1024 + (nt + 1) * NQ)
                            nc.tensor.matmul(
                                ps[:, nt * NQ:(nt + 1) * NQ],
                                w[ih][:, mh * 128:(mh + 1) * 128],
                                src[ih][:, cols],
                                start=(ih == 0), stop=(ih == 1),
                            )
                    return ps

                def qk_round(dst, w, bias, mh, nh, src):
                    ps = proj_round(w, mh, nh, src)
                    cols = slice(nh * 1024, (nh + 1) * 1024)
                    nc.vector.tensor_scalar_add(
                        out=dst[mh][:, cols], in0=ps[:], scalar1=bias[mh])

                def k_round(mh, nh):
                    # rows 0:64 -> head 2mh tile, rows 64:128 -> head 2mh+1;
                    # the other half of each per-head K tile is zeroed (x*0)
                    # so score matmuls can use the full K=128 contraction.
                    ps = proj_round(wkt, mh, nh, kin)
                    cols = slice(nh * 1024, (nh + 1) * 1024)
                    mult = mybir.AluOpType.mult
                    nc.vector.tensor_scalar_add(
                        out=k_sb[2 * mh][0:64, cols], in0=ps[0:64, :],
                        scalar1=bk[mh][0:64, :])
                    nc.vector.tensor_scalar(
                        out=k_sb[2 * mh][64:128, cols], in0=ps[64:128, :],
                        scalar1=0.0, scalar2=None, op0=mult)
                    nc.vector.tensor_scalar_add(
                        out=k_sb[2 * mh + 1][64:128, cols], in0=ps[64:128, :],
                        scalar1=bk[mh][64:128, :])
                    nc.vector.tensor_scalar(
                        out=k_sb[2 * mh + 1][0:64, cols], in0=ps[0:64, :],
                        scalar1=0.0, scalar2=None, op0=mult)

                for nh in range(2):
                    qk_round(q_sb, wqt, bq, 0, nh, qin)
                for nh in range(2):
                    k_round(0, nh)

                # Vt: [n, r] chunks with bias add + bf16 convert into vt layout
                for c in range(NCHUNKS):
                    pv = pp.tile([128, D], F32, tag="pvt", name="pvt")
                    for ih in range(2):
                        nc.tensor.matmul(
                            pv[:],
                            vin[ih][:, c * 128:(c + 1) * 128],
                            wvt[ih][:],
                            start=(ih == 0), stop=(ih == 1),
                        )
                    nc.vector.tensor_tensor(
                        out=vt[:, c, :, 0:HD],
                        in0=pv[:].rearrange("p (h d) -> p h d", h=H),
                        in1=bvb[:].rearrange("p (h d) -> p h d", h=H),
                        op=mybir.AluOpType.add,
                    )

                for nh in range(2):
                    qk_round(q_sb, wqt, bq, 1, nh, qin)
                for nh in range(2):
                    k_round(1, nh)

            # ---- attention units ----
            with (
                tc.tile_pool(name="sring", bufs=1, space="PSUM") as srp,
                tc.tile_pool(name="xacc", bufs=2, space="PSUM") as xap,
            ):
                sring = srp.tile([128, RING, NQ], F32, tag="s", name="s")
                NG = NUNITS * NCHUNKS  # 256 global chunks

                def emit_S(g):
                    u, c = divmod(g, NCHUNKS)
                    h, qj = divmod(u, N // NQ)
                    th = h // 2
                    nc.tensor.matmul(
                        sring[:, g % RING, :],
                        k_sb[h][:, c * 128:(c + 1) * 128],
                        q_sb[th][:, qj * NQ:(qj + 1) * NQ],
                        start=True, stop=True, skip_group_check=True,
                    )

                def emit_exp(gs):  # gs: consecutive global chunks, same ring block
                    # split on E-ring wraparound
                    i = 0
                    while i < len(gs):
                        es = gs[i] % ERING
                        k = 1
                        while (i + k < len(gs)) and (es + k < ERING):
                            k += 1
                        s0 = gs[i] % RING
                        nc.scalar.activation(
                            out=e_ring[:, es:es + k, :],
                            in_=sring[:, s0:s0 + k, :],
                            func=Exp,
                        )
                        i += k

                xaccs = {}

                def emit_PV(g):
                    u, c = divmod(g, NCHUNKS)
                    h = u // (N // NQ)
                    if c == 0:
                        xaccs[u] = xap.tile([HD + 1, NQ], F32, tag="xa", name="xa")
                    nc.tensor.matmul(
                        xaccs[u][:],
                        vt[:, c, h, :],
                        e_ring[:, g % ERING, :],
                        start=(c == 0), stop=(c == NCHUNKS - 1),
                        skip_group_check=True,
                    )

                def emit_norm(u):
                    h, qj = divmod(u, N // NQ)
                    th, hp = divmod(h, 2)
                    xa = xaccs.pop(u)
                    zrow = normp.tile([1, NQ], F32, tag="zrow", name="zrow")
                    nc.vector.tensor_copy(out=zrow[:], in_=xa[HD:HD + 1, :])
                    zrec = normp.tile([1, NQ], F32, tag="zrec", name="zrec")
                    nc.vector.reciprocal_approx_fast(out=zrec[:], in_=zrow[:])
                    zb = normp.tile([64, NQ], F32, tag="zb", name="zb")
                    nc.gpsimd.partition_broadcast(zb[:], zrec[:])
                    nc.vector.tensor_tensor(
                        out=x_sb[th][hp * 64:(hp + 1) * 64, qj * NQ:(qj + 1) * NQ],
                        in0=xa[0:HD, :],
                        in1=zb[:],
                        op=mybir.AluOpType.mult,
                    )

                # triad-pipelined emission: exp(T) directly after S(T) so no
                # later sring writer precedes it; PV(T-1) trails one triad.
                triads = [list(range(t * 3, min(t * 3 + 3, NG)))
                          for t in range((NG + 2) // 3)]
                for t, chunks in enumerate(triads):
                    for g in chunks:
                        emit_S(g)
                    emit_exp(chunks)
                    if t >= 1:
                        for g in triads[t - 1]:
                            emit_PV(g)
                            if g % NCHUNKS == NCHUNKS - 1:
                                emit_norm(g // NCHUNKS)
                for g in triads[-1]:
                    emit_PV(g)
                    if g % NCHUNKS == NCHUNKS - 1:
                        emit_norm(g // NCHUNKS)

            # ---- output projection ----
            with tc.tile_pool(name="po", bufs=2, space="PSUM") as pop:
                for mh in range(2):
                    for nh in range(2):
                        ps = pop.tile([128, 1024], F32, tag="po", name="po")
                        for ih in range(2):
                            for nt in range(2):
                                cols = slice(nh * 1024 + nt * NQ,
                                             nh * 1024 + (nt + 1) * NQ)
                                nc.tensor.matmul(
                                    ps[:, nt * NQ:(nt + 1) * NQ],
                                    wmt[ih][:, mh * 128:(mh + 1) * 128],
                                    x_sb[ih][:, cols],
                                    start=(ih == 0), stop=(ih == 1),
                                )
                        cols = slice(nh * 1024, (nh + 1) * 1024)
                        nc.vector.tensor_scalar_add(
                            out=o_sb[mh][:, cols], in0=ps[:], scalar1=bm[mh])
                    nc.sync.dma_start(
                        out=d_out[mh * 128:(mh + 1) * 128, :], in_=o_sb[mh][:])

    nc.finalize()
    return nc


def _get_nc():
    if "nc" not in _CACHE:
        _CACHE["nc"] = _build_nc()
    return _CACHE["nc"]


def _prep_host(Wq, bq, Wk, bk, Wv, bv, Wm, bm):
    r = np.arange(D)
    perm = (r % HD) * H + (r // HD)  # head-blocked row r -> original channel o
    s = np.float32(1.0 / np.sqrt(HD))
    f32 = np.float32
    wqt = np.ascontiguousarray((Wq[perm, :] * s).T, dtype=f32)
    bq_p = np.ascontiguousarray((bq[perm] * s)[:, None], dtype=f32)
    wkt = np.ascontiguousarray(Wk[perm, :].T, dtype=f32)
    bk_p = np.ascontiguousarray(bk[perm][:, None], dtype=f32)
    wvt = np.ascontiguousarray(Wv[perm, :].T, dtype=f32)
    bvb = np.ascontiguousarray(np.tile(bv[perm][None, :], (128, 1)), dtype=f32)
    wmt = np.ascontiguousarray(Wm[:, perm].T, dtype=f32)
    bm_p = np.ascontiguousarray(bm[:, None], dtype=f32)
    return dict(wqt=wqt, bq=bq_p, wkt=wkt, bk=bk_p, wvt=wvt, bvb=bvb,
                wmt=wmt, bm=bm_p)


def _run(inputs, trace=False):
    from concourse.bass_utils import run_bass_kernel_spmd

    query = np.ascontiguousarray(np.asarray(inputs["query"], dtype=np.float32))
    key = np.ascontiguousarray(np.asarray(inputs["key"], dtype=np.float32))
    value = np.ascontiguousarray(np.asarray(inputs["value"], dtype=np.float32))
    w = _prep_host(
        np.asarray(inputs["Wq"], np.float32), np.asarray(inputs["bq"], np.float32),
        np.asarray(inputs["Wk"], np.float32), np.asarray(inputs["bk"], np.float32),
        np.asarray(inputs["Wv"], np.float32), np.asarray(inputs["bv"], np.float32),
        np.asarray(inputs["Wm"], np.float32), np.asarray(inputs["bm"], np.float32),
    )
    in_maps = []
    for b in range(B):
        m = dict(w)
        m["query"] = np.ascontiguousarray(query[b])
        m["key"] = np.ascontiguousarray(key[b])
        m["value"] = np.ascontiguousarray(value[b])
        in_maps.append(m)
    nc = _get_nc()
    res = run_bass_kernel_spmd(nc, in_maps, core_ids=list(range(B)), trace=trace)
    out = np.stack([r["out"] for r in res.results], axis=0)
    return out, res


def kernel(**inputs):
    out, _ = _run(inputs, trace=False)
    return out


if __name__ == "__main__":
    rng = np.random.default_rng(0)
    s = 1.0 / np.sqrt(D)
    inputs = {
        "query": rng.standard_normal((B, D, N), dtype=np.float32),
        "key": rng.standard_normal((B, D, N), dtype=np.float32),
        "value": rng.standard_normal((B, D, N), dtype=np.float32),
        "Wq": rng.standard_normal((D, D), dtype=np.float32) * s,
        "bq": rng.standard_normal((D,), dtype=np.float32) * 0.01,
        "Wk": rng.standard_normal((D, D), dtype=np.float32) * s,
        "bk": rng.standard_normal((D,), dtype=np.float32) * 0.01,
        "Wv": rng.standard_normal((D, D), dtype=np.float32) * s,
        "bv": rng.standard_normal((D,), dtype=np.float32) * 0.01,
        "Wm": rng.standard_normal((D, D), dtype=np.float32) * s,
        "bm": rng.standard_normal((D,), dtype=np.float32) * 0.01,
    }
    out = kernel(**inputs)
    # numpy reference
    def proj(x, W, b):
        return np.einsum("oi,bin->bon", W, x) + b[None, :, None]
    q = proj(inputs["query"], inputs["Wq"], inputs["bq"]).reshape(B, HD, H, N)
    k = proj(inputs["key"], inputs["Wk"], inputs["bk"]).reshape(B, HD, H, N)
    v = proj(inputs["value"], inputs["Wv"], inputs["bv"]).reshape(B, HD, H, N)
    sc = np.einsum("bdhn,bdhm->bhnm", q, k) / np.sqrt(HD)
    sc = sc - sc.max(axis=-1, keepdims=True)
    p = np.exp(sc)
    p /= p.sum(axis=-1, keepdims=True)
    x = np.einsum("bhnm,bdhm->bdhn", p, v).reshape(B, D, N)
    ref = proj(x, inputs["Wm"], inputs["bm"])
    err = np.abs(out - ref)
    scale = np.abs(ref).max()
    print("abs err max:", err.max(), "scaled:", err.max() / scale)
    rel = np.linalg.norm(out - ref) / np.linalg.norm(ref)
    print("fro rel err:", rel)


# revision 9
# speedup vs baseline: 2.2908x; 1.5171x over previous
"""MultiHeadedAttention Trainium2 Bass kernel.

Full inputs -> full outputs. Shards batch (B=8) across 8 NeuronCores,
one batch element per core. Self-contained: hardcodes all shapes.

Math per core (batch item b):
  q = Wq @ query + bq  (channels o = d*4 + h permuted to head-blocked r = h*64 + d,
                        1/sqrt(64) folded into Wq/bq)
  k = Wk @ key + bk
  Vt[n, r] = (Wv @ value + bv)^T   (computed directly in transposed layout)
  per head h: S^T[m, n] = k_h^T-chunks x q_h ; E = exp(S^T) (no max subtraction:
              scores ~ N(0,1), exp is safe in fp32)
  x'[d, n]  = sum_m Vt_aug[m, d] * E[m, n]  with Vt_aug's 65th column = ones
              so row 64 of x' = softmax denominator Z[n]
  X[r, n]   = x'[d, n] / Z[n]
  out = Wm @ X + bm   (Wm columns pre-permuted to consume head-blocked X)
"""

import numpy as np

B = 8
D = 256
N = 2048
H = 4
HD = 64
NQ = 512            # unit column width (n-quarter)
NUNITS = H * (N // NQ)   # 16 units of (head, n-quarter)
NCHUNKS = 16        # m-chunks of 128 per unit
RING = 6            # psum score ring slots of [128, NQ]
ERING = 32          # E ring slots of [128, NQ] (2 units worth)

_CACHE = {}


def _build_nc():
    import concourse.bacc as bacc
    import concourse.mybir as mybir
    import concourse.tile as tile

    F32 = mybir.dt.float32
    F32R = mybir.dt.float32r
    BF16 = mybir.dt.bfloat16
    Exp = mybir.ActivationFunctionType.Exp

    nc = bacc.Bacc("TRN2", target_bir_lowering=False, debug=False, num_devices=B)

    # DRAM I/O (per-core shapes)
    d_q = nc.dram_tensor("query", [D, N], F32, kind="ExternalInput")
    d_k = nc.dram_tensor("key", [D, N], F32, kind="ExternalInput")
    d_v = nc.dram_tensor("value", [D, N], F32, kind="ExternalInput")
    d_wqt = nc.dram_tensor("wqt", [D, D], F32, kind="ExternalInput")
    d_wkt = nc.dram_tensor("wkt", [D, D], F32, kind="ExternalInput")
    d_wvt = nc.dram_tensor("wvt", [D, D], F32, kind="ExternalInput")
    d_wmt = nc.dram_tensor("wmt", [D, D], F32, kind="ExternalInput")
    d_bq = nc.dram_tensor("bq", [D, 1], F32, kind="ExternalInput")
    d_bk = nc.dram_tensor("bk", [D, 1], F32, kind="ExternalInput")
    d_bvb = nc.dram_tensor("bvb", [128, D], F32, kind="ExternalInput")
    d_bm = nc.dram_tensor("bm", [D, 1], F32, kind="ExternalInput")
    d_out = nc.dram_tensor("out", [D, N], F32, kind="ExternalOutput")

    with tile.TileContext(nc) as tc:
        with (
            tc.tile_pool(name="pers", bufs=1) as pers,
            tc.tile_pool(name="epool", bufs=1) as epool,
            tc.tile_pool(name="norm", bufs=3) as normp,
        ):
            # ---- persistent SBUF tiles ----
            qin = [pers.tile([128, N], F32R, tag=f"qin{i}", name=f"qin{i}") for i in range(2)]
            kin = [pers.tile([128, N], F32R, tag=f"kin{i}", name=f"kin{i}") for i in range(2)]
            vin = [pers.tile([128, N], F32R, tag=f"vin{i}", name=f"vin{i}") for i in range(2)]
            wqt = [pers.tile([128, D], F32R, tag=f"wqt{i}", name=f"wqt{i}") for i in range(2)]
            wkt = [pers.tile([128, D], F32R, tag=f"wkt{i}", name=f"wkt{i}") for i in range(2)]
            wvt = [pers.tile([128, D], F32R, tag=f"wvt{i}", name=f"wvt{i}") for i in range(2)]
            wmt = [pers.tile([128, D], F32R, tag=f"wmt{i}", name=f"wmt{i}") for i in range(2)]
            bq = [pers.tile([128, 1], F32, tag=f"bq{i}", name=f"bq{i}") for i in range(2)]
            bk = [pers.tile([128, 1], F32, tag=f"bk{i}", name=f"bk{i}") for i in range(2)]
            bm = [pers.tile([128, 1], F32, tag=f"bm{i}", name=f"bm{i}") for i in range(2)]
            bvb = pers.tile([128, D], F32, tag="bvb", name="bvb")
            q_sb = [pers.tile([128, N], F32R, tag=f"q{i}", name=f"q{i}") for i in range(2)]
            # K stored per head in zero-padded full-height tiles so the score
            # matmuls have K=128 (K=64 matmuls never HAM-warm and run ~2.4x slow)
            k_sb = [pers.tile([128, N], F32R, tag=f"k{i}", name=f"k{i}") for i in range(4)]
            x_sb = [pers.tile([128, N], F32R, tag=f"x{i}", name=f"x{i}") for i in range(2)]
            o_sb = [pers.tile([128, N], F32, tag=f"o{i}", name=f"o{i}") for i in range(2)]
            vt = pers.tile([128, NCHUNKS, H, HD + 1], BF16, tag="vt", name="vt")
            e_ring = epool.tile([128, ERING, NQ], BF16, tag="E", name="E")
            warm = pers.tile([1, 8], F32, tag="warm", name="warm")

            # ---- warm up the exp table on ACT as early as possible ----
            nc.vector.memset(warm[:], 0.0)
            nc.scalar.activation(out=warm[:], in_=warm[:], func=Exp)

            # ---- input DMAs ----
            for i in range(2):
                rows = slice(i * 128, (i + 1) * 128)
                nc.sync.dma_start(out=qin[i], in_=d_q[rows, :].bitcast(F32R))
                nc.sync.dma_start(out=kin[i], in_=d_k[rows, :].bitcast(F32R))
                nc.sync.dma_start(out=vin[i], in_=d_v[rows, :].bitcast(F32R))
                nc.sync.dma_start(out=wqt[i], in_=d_wqt[rows, :].bitcast(F32R))
                nc.sync.dma_start(out=wkt[i], in_=d_wkt[rows, :].bitcast(F32R))
                nc.sync.dma_start(out=wvt[i], in_=d_wvt[rows, :].bitcast(F32R))
                nc.sync.dma_start(out=wmt[i], in_=d_wmt[rows, :].bitcast(F32R))
                nc.sync.dma_start(out=bq[i], in_=d_bq[rows, :])
                nc.sync.dma_start(out=bk[i], in_=d_bk[rows, :])
                nc.sync.dma_start(out=bm[i], in_=d_bm[rows, :])
            nc.sync.dma_start(out=bvb, in_=d_bvb[:, :])

            # ones columns of vt (data columns overwritten below)
            nc.gpsimd.memset(vt[:, :, :, HD], 1.0)

            # ---- projections ----
            with tc.tile_pool(name="pp", bufs=2, space="PSUM") as pp:
                # Q and K: head-blocked [256, 2048], rows r = h*64+d.
                # mh0 first so heads 0/1 unblock the attention units early.
                def proj_round(w, mh, nh, src):
                    ps = pp.tile([128, 1024], F32, tag="pqk", name="pqk")
                    for ih in range(2):
                        for nt in range(2):
                            cols = slice(nh * 1024 + nt * NQ, nh * 1024 + (nt + 1) * NQ)
                            nc.tensor.matmul(
                                ps[:, nt * NQ:(nt + 1) * NQ],
                                w[ih][:, mh * 128:(mh + 1) * 128],
                                src[ih][:, cols],
                                start=(ih == 0), stop=(ih == 1),
                            )
                    return ps

                def qk_round(dst, w, bias, mh, nh, src):
                    ps = proj_round(w, mh, nh, src)
                    cols = slice(nh * 1024, (nh + 1) * 1024)
                    nc.vector.tensor_scalar_add(
                        out=dst[mh][:, cols], in0=ps[:], scalar1=bias[mh])

                def k_round(mh, nh):
                    # rows 0:64 -> head 2mh tile, rows 64:128 -> head 2mh+1;
                    # the other half of each per-head K tile is zeroed (x*0)
                    # so score matmuls can use the full K=128 contraction.
                    ps = proj_round(wkt, mh, nh, kin)
                    cols = slice(nh * 1024, (nh + 1) * 1024)
                    mult = mybir.AluOpType.mult
                    nc.vector.tensor_scalar_add(
                        out=k_sb[2 * mh][0:64, cols], in0=ps[0:64, :],
                        scalar1=bk[mh][0:64, :])
                    nc.vector.tensor_scalar(
                        out=k_sb[2 * mh][64:128, cols], in0=ps[64:128, :],
                        scalar1=0.0, scalar2=None, op0=mult)
                    nc.vector.tensor_scalar_add(
                        out=k_sb[2 * mh + 1][64:128, cols], in0=ps[64:128, :],
                        scalar1=bk[mh][64:128, :])
                    nc.vector.tensor_scalar(
                        out=k_sb[2 * mh + 1][0:64, cols], in0=ps[0:64, :],
                        scalar1=0.0, scalar2=None, op0=mult)

                for nh in range(2):
                    qk_round(q_sb, wqt, bq, 0, nh, qin)
                for nh in range(2):
                    k_round(0, nh)

                # Vt: [n, r] chunks with bias add + bf16 convert into vt layout
                for c in range(NCHUNKS):
                    pv = pp.tile([128, D], F32, tag="pvt", name="pvt")
                    for ih in range(2):
                        nc.tensor.matmul(
                            pv[:],
                            vin[ih][:, c * 128:(c + 1) * 128],
                            wvt[ih][:],
                            start=(ih == 0), stop=(ih == 1),
                        )
                    nc.vector.tensor_tensor(
                        out=vt[:, c, :, 0:HD],
                        in0=pv[:].rearrange("p (h d) -> p h d", h=H),
                        in1=bvb[:].rearrange("p (h d) -> p h d", h=H),
                        op=mybir.AluOpType.add,
                    )

                for nh in range(2):
                    qk_round(q_sb, wqt, bq, 1, nh, qin)
                for nh in range(2):
                    k_round(1, nh)

            # ---- attention units ----
            # Ping-pong separate PSUM/SBUF tensors per triad: Tile tracks
            # dependencies per-tensor, so a shared ring tensor creates false
            # WAR serialization between S-writes and exp-reads.
            with (
                tc.tile_pool(name="sring", bufs=1, space="PSUM") as srp,
                tc.tile_pool(name="xacc", bufs=2, space="PSUM") as xap,
            ):
                sr_ab = [srp.tile([128, 3, NQ], F32, tag=f"s{i}", name=f"s{i}")
                         for i in range(2)]
                e_ab = [epool.tile([128, 3, NQ], BF16, tag=f"E{i}", name=f"E{i}")
                        for i in range(4)]
                NG = NUNITS * NCHUNKS  # 256 global chunks

                def emit_S(g):
                    u, c = divmod(g, NCHUNKS)
                    h, qj = divmod(u, N // NQ)
                    th = h // 2
                    t, p = divmod(g, 3)
                    nc.tensor.matmul(
                        sr_ab[t % 2][:, p, :],
                        k_sb[h][:, c * 128:(c + 1) * 128],
                        q_sb[th][:, qj * NQ:(qj + 1) * NQ],
                        start=True, stop=True, skip_group_check=True,
                    )

                def emit_exp(t, nch):
                    nc.scalar.activation(
                        out=e_ab[t % 4][:, 0:nch, :],
                        in_=sr_ab[t % 2][:, 0:nch, :],
                        func=Exp,
                    )

                xaccs = {}

                def emit_PV(g):
                    u, c = divmod(g, NCHUNKS)
                    h = u // (N // NQ)
                    t, p = divmod(g, 3)
                    if c == 0:
                        xaccs[u] = xap.tile([HD + 1, NQ], F32, tag="xa",
                                            name="xa")
                    nc.tensor.matmul(
                        xaccs[u][:],
                        vt[:, c, h, :],
                        e_ab[t % 4][:, p, :],
                        start=(c == 0), stop=(c == NCHUNKS - 1),
                        skip_group_check=True,
                    )

                def emit_norm(u):
                    h, qj = divmod(u, N // NQ)
                    th, hp = divmod(h, 2)
                    xa = xaccs.pop(u)
                    zrow = normp.tile([1, NQ], F32, tag="zrow", name="zrow")
                    nc.vector.tensor_copy(out=zrow[:], in_=xa[HD:HD + 1, :])
                    zrec = normp.tile([1, NQ], F32, tag="zrec", name="zrec")
                    nc.vector.reciprocal_approx_fast(out=zrec[:], in_=zrow[:])
                    zb = normp.tile([64, NQ], F32, tag="zb", name="zb")
                    nc.gpsimd.partition_broadcast(zb[:], zrec[:])
                    nc.vector.tensor_tensor(
                        out=x_sb[th][hp * 64:(hp + 1) * 64, qj * NQ:(qj + 1) * NQ],
                        in0=xa[0:HD, :],
                        in1=zb[:],
                        op=mybir.AluOpType.mult,
                    )

                def emit_pv_triad(chunks):
                    for g in chunks:
                        emit_PV(g)
                        if g % NCHUNKS == NCHUNKS - 1:
                            emit_norm(g // NCHUNKS)

                # emission per triad T: S(T); PV(T-2); exp(T).
                triads = [list(range(t * 3, min(t * 3 + 3, NG)))
                          for t in range((NG + 2) // 3)]
                for t, chunks in enumerate(triads):
                    for g in chunks:
                        emit_S(g)
                    if t >= 2:
                        emit_pv_triad(triads[t - 2])
                    emit_exp(t, len(chunks))
                emit_pv_triad(triads[-2])
                emit_pv_triad(triads[-1])

            # ---- output projection ----
            with tc.tile_pool(name="po", bufs=2, space="PSUM") as pop:
                for mh in range(2):
                    for nh in range(2):
                        ps = pop.tile([128, 1024], F32, tag="po", name="po")
                        for ih in range(2):
                            for nt in range(2):
                                cols = slice(nh * 1024 + nt * NQ,
                                             nh * 1024 + (nt + 1) * NQ)
                                nc.tensor.matmul(
                                    ps[:, nt * NQ:(nt + 1) * NQ],
                                    wmt[ih][:, mh * 128:(mh + 1) * 128],
                                    x_sb[ih][:, cols],
                                    start=(ih == 0), stop=(ih == 1),
                                )
                        cols = slice(nh * 1024, (nh + 1) * 1024)
                        nc.vector.tensor_scalar_add(
                            out=o_sb[mh][:, cols], in0=ps[:], scalar1=bm[mh])
                    nc.sync.dma_start(
                        out=d_out[mh * 128:(mh + 1) * 128, :], in_=o_sb[mh][:])

    nc.finalize()
    return nc


def _get_nc():
    if "nc" not in _CACHE:
        _CACHE["nc"] = _build_nc()
    return _CACHE["nc"]


def _prep_host(Wq, bq, Wk, bk, Wv, bv, Wm, bm):
    r = np.arange(D)
    perm = (r % HD) * H + (r // HD)  # head-blocked row r -> original channel o
    s = np.float32(1.0 / np.sqrt(HD))
    f32 = np.float32
    wqt = np.ascontiguousarray((Wq[perm, :] * s).T, dtype=f32)
    bq_p = np.ascontiguousarray((bq[perm] * s)[:, None], dtype=f32)
    wkt = np.ascontiguousarray(Wk[perm, :].T, dtype=f32)
    bk_p = np.ascontiguousarray(bk[perm][:, None], dtype=f32)
    wvt = np.ascontiguousarray(Wv[perm, :].T, dtype=f32)
    bvb = np.ascontiguousarray(np.tile(bv[perm][None, :], (128, 1)), dtype=f32)
    wmt = np.ascontiguousarray(Wm[:, perm].T, dtype=f32)
    bm_p = np.ascontiguousarray(bm[:, None], dtype=f32)
    return dict(wqt=wqt, bq=bq_p, wkt=wkt, bk=bk_p, wvt=wvt, bvb=bvb,
                wmt=wmt, bm=bm_p)


def _run(inputs, trace=False):
    from concourse.bass_utils import run_bass_kernel_spmd

    query = np.ascontiguousarray(np.asarray(inputs["query"], dtype=np.float32))
    key = np.ascontiguousarray(np.asarray(inputs["key"], dtype=np.float32))
    value = np.ascontiguousarray(np.asarray(inputs["value"], dtype=np.float32))
    w = _prep_host(
        np.asarray(inputs["Wq"], np.float32), np.asarray(inputs["bq"], np.float32),
        np.asarray(inputs["Wk"], np.float32), np.asarray(inputs["bk"], np.float32),
        np.asarray(inputs["Wv"], np.float32), np.asarray(inputs["bv"], np.float32),
        np.asarray(inputs["Wm"], np.float32), np.asarray(inputs["bm"], np.float32),
    )
    in_maps = []
    for b in range(B):
        m = dict(w)
        m["query"] = np.ascontiguousarray(query[b])
        m["key"] = np.ascontiguousarray(key[b])
        m["value"] = np.ascontiguousarray(value[b])
        in_maps.append(m)
    nc = _get_nc()
    res = run_bass_kernel_spmd(nc, in_maps, core_ids=list(range(B)), trace=trace)
    out = np.stack([r["out"] for r in res.results], axis=0)
    return out, res


def kernel(**inputs):
    out, _ = _run(inputs, trace=False)
    return out


if __name__ == "__main__":
    rng = np.random.default_rng(0)
    s = 1.0 / np.sqrt(D)
    inputs = {
        "query": rng.standard_normal((B, D, N), dtype=np.float32),
        "key": rng.standard_normal((B, D, N), dtype=np.float32),
        "value": rng.standard_normal((B, D, N), dtype=np.float32),
        "Wq": rng.standard_normal((D, D), dtype=np.float32) * s,
        "bq": rng.standard_normal((D,), dtype=np.float32) * 0.01,
        "Wk": rng.standard_normal((D, D), dtype=np.float32) * s,
        "bk": rng.standard_normal((D,), dtype=np.float32) * 0.01,
        "Wv": rng.standard_normal((D, D), dtype=np.float32) * s,
        "bv": rng.standard_normal((D,), dtype=np.float32) * 0.01,
        "Wm": rng.standard_normal((D, D), dtype=np.float32) * s,
        "bm": rng.standard_normal((D,), dtype=np.float32) * 0.01,
    }
    out = kernel(**inputs)
    # numpy reference
    def proj(x, W, b):
        return np.einsum("oi,bin->bon", W, x) + b[None, :, None]
    q = proj(inputs["query"], inputs["Wq"], inputs["bq"]).reshape(B, HD, H, N)
    k = proj(inputs["key"], inputs["Wk"], inputs["bk"]).reshape(B, HD, H, N)
    v = proj(inputs["value"], inputs["Wv"], inputs["bv"]).reshape(B, HD, H, N)
    sc = np.einsum("bdhn,bdhm->bhnm", q, k) / np.sqrt(HD)
    sc = sc - sc.max(axis=-1, keepdims=True)
    p = np.exp(sc)
    p /= p.sum(axis=-1, keepdims=True)
    x = np.einsum("bhnm,bdhm->bdhn", p, v).reshape(B, D, N)
    ref = proj(x, inputs["Wm"], inputs["bm"])
    err = np.abs(out - ref)
    scale = np.abs(ref).max()
    print("abs err max:", err.max(), "scaled:", err.max() / scale)
    rel = np.linalg.norm(out - ref) / np.linalg.norm(ref)
    print("fro rel err:", rel)


# revision 12
# speedup vs baseline: 2.4620x; 1.0747x over previous
"""MultiHeadedAttention Trainium2 Bass kernel.

Full inputs -> full outputs. Shards batch (B=8) across 8 NeuronCores,
one batch element per core. Self-contained: hardcodes all shapes.

Math per core (batch item b):
  q = Wq @ query + bq  (channels o = d*4 + h permuted to head-blocked r = h*64 + d,
                        1/sqrt(64) folded into Wq/bq)
  k = Wk @ key + bk
  Vt[n, r] = (Wv @ value + bv)^T   (computed directly in transposed layout)
  per head h: S^T[m, n] = k_h^T-chunks x q_h ; E = exp(S^T) (no max subtraction:
              scores ~ N(0,1), exp is safe in fp32)
  x'[d, n]  = sum_m Vt_aug[m, d] * E[m, n]  with Vt_aug's 65th column = ones
              so row 64 of x' = softmax denominator Z[n]
  X[r, n]   = x'[d, n] / Z[n]
  out = Wm @ X + bm   (Wm columns pre-permuted to consume head-blocked X)
"""

import numpy as np

B = 8
D = 256
N = 2048
H = 4
HD = 64
NQ = 512            # unit column width (n-quarter)
NUNITS = H * (N // NQ)   # 16 units of (head, n-quarter)
NCHUNKS = 16        # m-chunks of 128 per unit
RING = 6            # psum score ring slots of [128, NQ]
ERING = 32          # E ring slots of [128, NQ] (2 units worth)

_CACHE = {}


def _build_nc():
    import concourse.bacc as bacc
    import concourse.mybir as mybir
    import concourse.tile as tile

    F32 = mybir.dt.float32
    F32R = mybir.dt.float32r
    BF16 = mybir.dt.bfloat16
    Exp = mybir.ActivationFunctionType.Exp

    nc = bacc.Bacc("TRN2", target_bir_lowering=False, debug=False, num_devices=B)

    # DRAM I/O (per-core shapes)
    d_q = nc.dram_tensor("query", [D, N], F32, kind="ExternalInput")
    d_k = nc.dram_tensor("key", [D, N], F32, kind="ExternalInput")
    d_v = nc.dram_tensor("value", [D, N], F32, kind="ExternalInput")
    d_wqt = nc.dram_tensor("wqt", [D, D], F32, kind="ExternalInput")
    d_wkt = nc.dram_tensor("wkt", [D, D], F32, kind="ExternalInput")
    d_wvt = nc.dram_tensor("wvt", [D, D], F32, kind="ExternalInput")
    d_wmt = nc.dram_tensor("wmt", [D, D], F32, kind="ExternalInput")
    d_bq = nc.dram_tensor("bq", [D, 1], F32, kind="ExternalInput")
    d_bk = nc.dram_tensor("bk", [D, 1], F32, kind="ExternalInput")
    d_bm = nc.dram_tensor("bm", [D, 1], F32, kind="ExternalInput")
    d_out = nc.dram_tensor("out", [D, N], F32, kind="ExternalOutput")

    with tile.TileContext(nc) as tc:
        with (
            tc.tile_pool(name="pers", bufs=1) as pers,
            tc.tile_pool(name="epool", bufs=1) as epool,
            tc.tile_pool(name="norm", bufs=3) as normp,
        ):
            # ---- persistent SBUF tiles ----
            qin = [pers.tile([128, N], F32R, tag=f"qin{i}", name=f"qin{i}") for i in range(2)]
            kin = [pers.tile([128, N], F32R, tag=f"kin{i}", name=f"kin{i}") for i in range(2)]
            vin = [pers.tile([128, N], F32R, tag=f"vin{i}", name=f"vin{i}") for i in range(2)]
            wqt = [pers.tile([128, D], F32R, tag=f"wqt{i}", name=f"wqt{i}") for i in range(2)]
            wkt = [pers.tile([128, D], F32R, tag=f"wkt{i}", name=f"wkt{i}") for i in range(2)]
            wvt = [pers.tile([128, D], F32R, tag=f"wvt{i}", name=f"wvt{i}") for i in range(2)]
            wmt = [pers.tile([128, D], F32R, tag=f"wmt{i}", name=f"wmt{i}") for i in range(2)]
            bq = [pers.tile([128, 1], F32, tag=f"bq{i}", name=f"bq{i}") for i in range(2)]
            bk = [pers.tile([128, 1], F32, tag=f"bk{i}", name=f"bk{i}") for i in range(2)]
            bm = [pers.tile([128, 1], F32, tag=f"bm{i}", name=f"bm{i}") for i in range(2)]
            q_sb = [pers.tile([128, N], F32R, tag=f"q{i}", name=f"q{i}") for i in range(2)]
            # K stored per head in zero-padded full-height tiles so the score
            # matmuls have K=128 (K=64 matmuls never HAM-warm and run ~2.4x slow)
            k_sb = [pers.tile([128, N], F32R, tag=f"k{i}", name=f"k{i}") for i in range(4)]
            x_sb = [pers.tile([128, N], F32R, tag=f"x{i}", name=f"x{i}") for i in range(2)]
            o_sb = [pers.tile([128, N], F32, tag=f"o{i}", name=f"o{i}") for i in range(2)]
            vt = pers.tile([128, NCHUNKS, H, HD + 1], BF16, tag="vt", name="vt")
            warm = pers.tile([1, 8], F32, tag="warm", name="warm")

            # ---- warm up the exp table on ACT as early as possible ----
            nc.vector.memset(warm[:], 0.0)
            nc.scalar.activation(out=warm[:], in_=warm[:], func=Exp)

            # ---- input DMAs + projections, interleaved so the first
            # attention unit can start as early as possible ----
            zscr = pers.tile([64, N], F32, tag="zscr", name="zscr")

            def dma_in_chunks(dst, dsrc, nh):
                cols = slice(nh * 1024, (nh + 1) * 1024)
                for i in range(2):
                    rows = slice(i * 128, (i + 1) * 128)
                    nc.sync.dma_start(out=dst[i][:, cols],
                                      in_=dsrc[rows, cols].bitcast(F32R))

            # weights + biases first (small)
            for i in range(2):
                rows = slice(i * 128, (i + 1) * 128)
                nc.sync.dma_start(out=wqt[i], in_=d_wqt[rows, :].bitcast(F32R))
                nc.sync.dma_start(out=wkt[i], in_=d_wkt[rows, :].bitcast(F32R))
                nc.sync.dma_start(out=bq[i], in_=d_bq[rows, :])
                nc.sync.dma_start(out=bk[i], in_=d_bk[rows, :])
            dma_in_chunks(qin, d_q, 0)
            dma_in_chunks(kin, d_k, 0)
            for i in range(2):
                rows = slice(i * 128, (i + 1) * 128)
                nc.sync.dma_start(out=wvt[i], in_=d_wvt[rows, :].bitcast(F32R))
                nc.sync.dma_start(out=vin[i], in_=d_v[rows, :].bitcast(F32R))
            dma_in_chunks(qin, d_q, 1)
            dma_in_chunks(kin, d_k, 1)
            for i in range(2):
                rows = slice(i * 128, (i + 1) * 128)
                nc.sync.dma_start(out=wmt[i], in_=d_wmt[rows, :].bitcast(F32R))
                nc.sync.dma_start(out=bm[i], in_=d_bm[rows, :])

            # ones columns of vt (data columns written by ACT copies below)
            nc.gpsimd.memset(vt[:, :, :, HD], 1.0)
            # zero halves of the per-head K tiles, off the critical path
            # (memset on an f32r tile fails walrus codegen, so memset an f32
            # scratch and copy)
            nc.gpsimd.memset(zscr[:], 0.0)
            for h in range(H):
                hp = h % 2
                nc.vector.tensor_copy(
                    out=k_sb[h][(1 - hp) * 64:(2 - hp) * 64, :], in_=zscr[:])

            # ---- projections ----
            with tc.tile_pool(name="pp", bufs=2, space="PSUM") as pp:
                Ident = mybir.ActivationFunctionType.Identity

                def proj_round(w, mh, nh, src):
                    ps = pp.tile([128, 1024], F32, tag="pqk", name="pqk")
                    for ih in range(2):
                        for nt in range(2):
                            cols = slice(nh * 1024 + nt * NQ, nh * 1024 + (nt + 1) * NQ)
                            nc.tensor.matmul(
                                ps[:, nt * NQ:(nt + 1) * NQ],
                                w[ih][:, mh * 128:(mh + 1) * 128],
                                src[ih][:, cols],
                                start=(ih == 0), stop=(ih == 1),
                            )
                    return ps

                def qk_round(dst, w, bias, mh, nh, src, eng):
                    ps = proj_round(w, mh, nh, src)
                    cols = slice(nh * 1024, (nh + 1) * 1024)
                    if eng == "v":
                        nc.vector.tensor_scalar_add(
                            out=dst[mh][:, cols], in0=ps[:], scalar1=bias[mh])
                    else:
                        nc.scalar.activation(
                            out=dst[mh][:, cols], in_=ps[:], func=Ident,
                            bias=bias[mh], scale=1.0)

                def k_round(mh, nh):
                    # rows 0:64 -> head 2mh tile, rows 64:128 -> head 2mh+1
                    # (split between DVE and ACT to halve the copy latency)
                    ps = proj_round(wkt, mh, nh, kin)
                    cols = slice(nh * 1024, (nh + 1) * 1024)
                    nc.vector.tensor_scalar_add(
                        out=k_sb[2 * mh][0:64, cols], in0=ps[0:64, :],
                        scalar1=bk[mh][0:64, :])
                    nc.scalar.activation(
                        out=k_sb[2 * mh + 1][64:128, cols], in_=ps[64:128, :],
                        func=Ident, bias=bk[mh][64:128, :], scale=1.0)

                qk_round(q_sb, wqt, bq, 0, 0, qin, "v")
                k_round(0, 0)
                qk_round(q_sb, wqt, bq, 0, 1, qin, "a")
                k_round(0, 1)

                # Vt: [n, r] chunks, plain copy+bf16-convert on ACT
                # (V bias is folded into bm on the host)
                for c in range(NCHUNKS):
                    pv = pp.tile([128, D], F32, tag="pvt", name="pvt")
                    for ih in range(2):
                        nc.tensor.matmul(
                            pv[:],
                            vin[ih][:, c * 128:(c + 1) * 128],
                            wvt[ih][:],
                            start=(ih == 0), stop=(ih == 1),
                        )
                    eng = nc.vector if c % 2 else nc.scalar
                    if c % 2:
                        nc.vector.tensor_copy(
                            out=vt[:, c, :, 0:HD],
                            in_=pv[:].rearrange("p (h d) -> p h d", h=H))
                    else:
                        nc.scalar.copy(
                            out=vt[:, c, :, 0:HD],
                            in_=pv[:].rearrange("p (h d) -> p h d", h=H))

                qk_round(q_sb, wqt, bq, 1, 0, qin, "v")
                k_round(1, 0)
                qk_round(q_sb, wqt, bq, 1, 1, qin, "a")
                k_round(1, 1)

            # ---- attention units ----
            # Ping-pong separate PSUM/SBUF tensors per triad: Tile tracks
            # dependencies per-tensor, so a shared ring tensor creates false
            # WAR serialization between S-writes and exp-reads.
            with (
                tc.tile_pool(name="sring", bufs=1, space="PSUM") as srp,
                tc.tile_pool(name="xacc", bufs=2, space="PSUM") as xap,
            ):
                sr_ab = [srp.tile([128, 3, NQ], F32, tag=f"s{i}", name=f"s{i}")
                         for i in range(2)]
                e_ab = [epool.tile([128, 3, NQ], BF16, tag=f"E{i}", name=f"E{i}")
                        for i in range(4)]
                NG = NUNITS * NCHUNKS  # 256 global chunks

                def emit_S(g):
                    u, c = divmod(g, NCHUNKS)
                    h, qj = divmod(u, N // NQ)
                    th = h // 2
                    t, p = divmod(g, 3)
                    nc.tensor.matmul(
                        sr_ab[t % 2][:, p, :],
                        k_sb[h][:, c * 128:(c + 1) * 128],
                        q_sb[th][:, qj * NQ:(qj + 1) * NQ],
                        start=True, stop=True, skip_group_check=True,
                    )

                def emit_exp(t, nch):
                    nc.scalar.activation(
                        out=e_ab[t % 4][:, 0:nch, :],
                        in_=sr_ab[t % 2][:, 0:nch, :],
                        func=Exp,
                    )

                xaccs = {}

                def emit_PV(g):
                    u, c = divmod(g, NCHUNKS)
                    h = u // (N // NQ)
                    t, p = divmod(g, 3)
                    if c == 0:
                        xaccs[u] = xap.tile([HD + 1, NQ], F32, tag="xa",
                                            name="xa")
                    nc.tensor.matmul(
                        xaccs[u][:],
                        vt[:, c, h, :],
                        e_ab[t % 4][:, p, :],
                        start=(c == 0), stop=(c == NCHUNKS - 1),
                        skip_group_check=True,
                    )

                def emit_norm(u):
                    h, qj = divmod(u, N // NQ)
                    th, hp = divmod(h, 2)
                    xa = xaccs.pop(u)
                    zrow = normp.tile([1, NQ], F32, tag="zrow", name="zrow")
                    nc.vector.tensor_copy(out=zrow[:], in_=xa[HD:HD + 1, :])
                    zrec = normp.tile([1, NQ], F32, tag="zrec", name="zrec")
                    nc.vector.reciprocal_approx_fast(out=zrec[:], in_=zrow[:])
                    zb = normp.tile([64, NQ], F32, tag="zb", name="zb")
                    nc.gpsimd.partition_broadcast(zb[:], zrec[:])
                    nc.vector.tensor_tensor(
                        out=x_sb[th][hp * 64:(hp + 1) * 64, qj * NQ:(qj + 1) * NQ],
                        in0=xa[0:HD, :],
                        in1=zb[:],
                        op=mybir.AluOpType.mult,
                    )

                def emit_pv_triad(chunks):
                    for g in chunks:
                        emit_PV(g)
                        if g % NCHUNKS == NCHUNKS - 1:
                            emit_norm(g // NCHUNKS)

                # emission per triad T: S(T); PV(T-2); exp(T).
                triads = [list(range(t * 3, min(t * 3 + 3, NG)))
                          for t in range((NG + 2) // 3)]
                for t, chunks in enumerate(triads):
                    for g in chunks:
                        emit_S(g)
                    if t >= 2:
                        emit_pv_triad(triads[t - 2])
                    emit_exp(t, len(chunks))
                emit_pv_triad(triads[-2])
                emit_pv_triad(triads[-1])

            # ---- output projection ----
            with tc.tile_pool(name="po", bufs=2, space="PSUM") as pop:
                for mh in range(2):
                    for nh in range(2):
                        ps = pop.tile([128, 1024], F32, tag="po", name="po")
                        for ih in range(2):
                            for nt in range(2):
                                cols = slice(nh * 1024 + nt * NQ,
                                             nh * 1024 + (nt + 1) * NQ)
                                nc.tensor.matmul(
                                    ps[:, nt * NQ:(nt + 1) * NQ],
                                    wmt[ih][:, mh * 128:(mh + 1) * 128],
                                    x_sb[ih][:, cols],
                                    start=(ih == 0), stop=(ih == 1),
                                )
                        cols = slice(nh * 1024, (nh + 1) * 1024)
                        if nh % 2:
                            nc.vector.tensor_scalar_add(
                                out=o_sb[mh][:, cols], in0=ps[:], scalar1=bm[mh])
                        else:
                            nc.scalar.activation(
                                out=o_sb[mh][:, cols], in_=ps[:],
                                func=mybir.ActivationFunctionType.Identity,
                                bias=bm[mh], scale=1.0)
                    nc.sync.dma_start(
                        out=d_out[mh * 128:(mh + 1) * 128, :], in_=o_sb[mh][:])

    nc.finalize()
    return nc


def _get_nc():
    if "nc" not in _CACHE:
        _CACHE["nc"] = _build_nc()
    return _CACHE["nc"]


def _prep_host(Wq, bq, Wk, bk, Wv, bv, Wm, bm):
    r = np.arange(D)
    perm = (r % HD) * H + (r // HD)  # head-blocked row r -> original channel o
    s = np.float32(1.0 / np.sqrt(HD))
    f32 = np.float32
    wqt = np.ascontiguousarray((Wq[perm, :] * s).T, dtype=f32)
    bq_p = np.ascontiguousarray((bq[perm] * s)[:, None], dtype=f32)
    wkt = np.ascontiguousarray(Wk[perm, :].T, dtype=f32)
    bk_p = np.ascontiguousarray(bk[perm][:, None], dtype=f32)
    wvt = np.ascontiguousarray(Wv[perm, :].T, dtype=f32)
    wmt = np.ascontiguousarray(Wm[:, perm].T, dtype=f32)
    # V-bias folds into the output projection bias: X = X0 + bv (per row),
    # so out = Wm_hb @ X0 + (bm + Wm_hb @ bv_hb)
    bm_p = np.ascontiguousarray(
        (bm + Wm[:, perm] @ bv[perm])[:, None], dtype=f32)
    return dict(wqt=wqt, bq=bq_p, wkt=wkt, bk=bk_p, wvt=wvt,
                wmt=wmt, bm=bm_p)


def _run(inputs, trace=False):
    from concourse.bass_utils import run_bass_kernel_spmd

    query = np.ascontiguousarray(np.asarray(inputs["query"], dtype=np.float32))
    key = np.ascontiguousarray(np.asarray(inputs["key"], dtype=np.float32))
    value = np.ascontiguousarray(np.asarray(inputs["value"], dtype=np.float32))
    w = _prep_host(
        np.asarray(inputs["Wq"], np.float32), np.asarray(inputs["bq"], np.float32),
        np.asarray(inputs["Wk"], np.float32), np.asarray(inputs["bk"], np.float32),
        np.asarray(inputs["Wv"], np.float32), np.asarray(inputs["bv"], np.float32),
        np.asarray(inputs["Wm"], np.float32), np.asarray(inputs["bm"], np.float32),
    )
    in_maps = []
    for b in range(B):
        m = dict(w)
        m["query"] = np.ascontiguousarray(query[b])
        m["key"] = np.ascontiguousarray(key[b])
        m["value"] = np.ascontiguousarray(value[b])
        in_maps.append(m)
    nc = _get_nc()
    res = run_bass_kernel_spmd(nc, in_maps, core_ids=list(range(B)), trace=trace)
    out = np.stack([r["out"] for r in res.results], axis=0)
    return out, res


def kernel(**inputs):
    out, _ = _run(inputs, trace=False)
    return out


if __name__ == "__main__":
    rng = np.random.default_rng(0)
    s = 1.0 / np.sqrt(D)
    inputs = {
        "query": rng.standard_normal((B, D, N), dtype=np.float32),
        "key": rng.standard_normal((B, D, N), dtype=np.float32),
        "value": rng.standard_normal((B, D, N), dtype=np.float32),
        "Wq": rng.standard_normal((D, D), dtype=np.float32) * s,
        "bq": rng.standard_normal((D,), dtype=np.float32) * 0.01,
        "Wk": rng.standard_normal((D, D), dtype=np.float32) * s,
        "bk": rng.standard_normal((D,), dtype=np.float32) * 0.01,
        "Wv": rng.standard_normal((D, D), dtype=np.float32) * s,
        "bv": rng.standard_normal((D,), dtype=np.float32) * 0.01,
        "Wm": rng.standard_normal((D, D), dtype=np.float32) * s,
        "bm": rng.standard_normal((D,), dtype=np.float32) * 0.01,
    }
    out = kernel(**inputs)
    # numpy reference
    def proj(x, W, b):
        return np.einsum("oi,bin->bon", W, x) + b[None, :, None]
    q = proj(inputs["query"], inputs["Wq"], inputs["bq"]).reshape(B, HD, H, N)
    k = proj(inputs["key"], inputs["Wk"], inputs["bk"]).reshape(B, HD, H, N)
    v = proj(inputs["value"], inputs["Wv"], inputs["bv"]).reshape(B, HD, H, N)
    sc = np.einsum("bdhn,bdhm->bhnm", q, k) / np.sqrt(HD)
    sc = sc - sc.max(axis=-1, keepdims=True)
    p = np.exp(sc)
    p /= p.sum(axis=-1, keepdims=True)
    x = np.einsum("bhnm,bdhm->bdhn", p, v).reshape(B, D, N)
    ref = proj(x, inputs["Wm"], inputs["bm"])
    err = np.abs(out - ref)
    scale = np.abs(ref).max()
    print("abs err max:", err.max(), "scaled:", err.max() / scale)
    rel = np.linalg.norm(out - ref) / np.linalg.norm(ref)
    print("fro rel err:", rel)


# revision 13
# speedup vs baseline: 2.5652x; 1.0420x over previous
"""MultiHeadedAttention Trainium2 Bass kernel.

Full inputs -> full outputs. Shards batch (B=8) across 8 NeuronCores,
one batch element per core. Self-contained: hardcodes all shapes.

Math per core (batch item b):
  q = Wq @ query + bq  (channels o = d*4 + h permuted to head-blocked r = h*64 + d,
                        1/sqrt(64) folded into Wq/bq)
  k = Wk @ key + bk
  Vt[n, r] = (Wv @ value + bv)^T   (computed directly in transposed layout)
  per head h: S^T[m, n] = k_h^T-chunks x q_h ; E = exp(S^T) (no max subtraction:
              scores ~ N(0,1), exp is safe in fp32)
  x'[d, n]  = sum_m Vt_aug[m, d] * E[m, n]  with Vt_aug's 65th column = ones
              so row 64 of x' = softmax denominator Z[n]
  X[r, n]   = x'[d, n] / Z[n]
  out = Wm @ X + bm   (Wm columns pre-permuted to consume head-blocked X)
"""

import numpy as np

B = 8
D = 256
N = 2048
H = 4
HD = 64
NQ = 512            # unit column width (n-quarter)
NUNITS = H * (N // NQ)   # 16 units of (head, n-quarter)
NCHUNKS = 16        # m-chunks of 128 per unit
RING = 6            # psum score ring slots of [128, NQ]
ERING = 32          # E ring slots of [128, NQ] (2 units worth)

_CACHE = {}


def _build_nc():
    import concourse.bacc as bacc
    import concourse.mybir as mybir
    import concourse.tile as tile

    F32 = mybir.dt.float32
    F32R = mybir.dt.float32r
    BF16 = mybir.dt.bfloat16
    Exp = mybir.ActivationFunctionType.Exp
    Ident = mybir.ActivationFunctionType.Identity

    nc = bacc.Bacc("TRN2", target_bir_lowering=False, debug=False, num_devices=B)

    # DRAM I/O (per-core shapes)
    d_q = nc.dram_tensor("query", [D, N], F32, kind="ExternalInput")
    d_k = nc.dram_tensor("key", [D, N], F32, kind="ExternalInput")
    d_v = nc.dram_tensor("value", [D, N], F32, kind="ExternalInput")
    d_wqt = nc.dram_tensor("wqt", [D, D], F32, kind="ExternalInput")
    d_wkt = nc.dram_tensor("wkt", [D, D], F32, kind="ExternalInput")
    d_wvt = nc.dram_tensor("wvt", [D, D], F32, kind="ExternalInput")
    d_wmt = nc.dram_tensor("wmt", [D, D], F32, kind="ExternalInput")
    d_bq = nc.dram_tensor("bq", [D, 1], F32, kind="ExternalInput")
    d_bk = nc.dram_tensor("bk", [D, 1], F32, kind="ExternalInput")
    d_bm = nc.dram_tensor("bm", [D, 1], F32, kind="ExternalInput")
    d_out = nc.dram_tensor("out", [D, N], F32, kind="ExternalOutput")

    with tile.TileContext(nc) as tc:
        with (
            tc.tile_pool(name="pers", bufs=1) as pers,
            tc.tile_pool(name="epool", bufs=1) as epool,
            tc.tile_pool(name="norm", bufs=3) as normp,
            tc.tile_pool(name="mix", bufs=2, space="PSUM") as mix,
            tc.tile_pool(name="sring", bufs=1, space="PSUM") as srp,
        ):
            # ---- persistent SBUF tiles ----
            qin = [pers.tile([128, N], F32R, tag=f"qin{i}", name=f"qin{i}") for i in range(2)]
            kin = [pers.tile([128, N], F32R, tag=f"kin{i}", name=f"kin{i}") for i in range(2)]
            vin = [pers.tile([128, N], F32R, tag=f"vin{i}", name=f"vin{i}") for i in range(2)]
            wqt = [pers.tile([128, D], F32R, tag=f"wqt{i}", name=f"wqt{i}") for i in range(2)]
            wkt = [pers.tile([128, D], F32R, tag=f"wkt{i}", name=f"wkt{i}") for i in range(2)]
            wvt = [pers.tile([128, D], F32R, tag=f"wvt{i}", name=f"wvt{i}") for i in range(2)]
            wmt = [pers.tile([128, D], F32R, tag=f"wmt{i}", name=f"wmt{i}") for i in range(2)]
            bq = [pers.tile([128, 1], F32, tag=f"bq{i}", name=f"bq{i}") for i in range(2)]
            bk = [pers.tile([128, 1], F32, tag=f"bk{i}", name=f"bk{i}") for i in range(2)]
            bm = [pers.tile([128, 1], F32, tag=f"bm{i}", name=f"bm{i}") for i in range(2)]
            q_sb = [pers.tile([128, N], F32R, tag=f"q{i}", name=f"q{i}") for i in range(2)]
            # K per head in zero-padded full-height tiles: K=64 matmuls never
            # HAM-warm and run ~2.4x slow, so pad to K=128 with zero rows.
            k_sb = [pers.tile([128, N], F32R, tag=f"k{i}", name=f"k{i}") for i in range(4)]
            x_sb = [pers.tile([128, N], F32R, tag=f"x{i}", name=f"x{i}") for i in range(2)]
            o_sb = [pers.tile([128, N], F32, tag=f"o{i}", name=f"o{i}") for i in range(2)]
            vt = pers.tile([128, NCHUNKS, H, HD + 1], BF16, tag="vt", name="vt")
            zscr = pers.tile([64, N], F32, tag="zscr", name="zscr")
            warm = pers.tile([1, 8], F32, tag="warm", name="warm")
            # score ring: 2 ping-pong tensors (Tile deps are tensor-coarse)
            sr_ab = [srp.tile([128, 3, NQ], F32, tag=f"s{i}", name=f"s{i}")
                     for i in range(2)]
            e_ab = [epool.tile([128, 3, NQ], BF16, tag=f"E{i}", name=f"E{i}")
                    for i in range(4)]

            # ---- warm up the exp table on ACT as early as possible ----
            nc.vector.memset(warm[:], 0.0)
            nc.scalar.activation(out=warm[:], in_=warm[:], func=Exp)

            # ---- input DMAs, critical-path first ----
            def dma_half(dst, dsrc, nh):
                cols = slice(nh * 1024, (nh + 1) * 1024)
                for i in range(2):
                    rows = slice(i * 128, (i + 1) * 128)
                    nc.sync.dma_start(out=dst[i][:, cols],
                                      in_=dsrc[rows, cols].bitcast(F32R))

            for i in range(2):
                rows = slice(i * 128, (i + 1) * 128)
                nc.sync.dma_start(out=wqt[i], in_=d_wqt[rows, :].bitcast(F32R))
                nc.sync.dma_start(out=wkt[i], in_=d_wkt[rows, :].bitcast(F32R))
                nc.sync.dma_start(out=bq[i], in_=d_bq[rows, :])
                nc.sync.dma_start(out=bk[i], in_=d_bk[rows, :])
            dma_half(qin, d_q, 0)
            dma_half(kin, d_k, 0)
            for i in range(2):
                rows = slice(i * 128, (i + 1) * 128)
                nc.sync.dma_start(out=wvt[i], in_=d_wvt[rows, :].bitcast(F32R))
                nc.sync.dma_start(out=vin[i], in_=d_v[rows, :].bitcast(F32R))
            dma_half(qin, d_q, 1)
            dma_half(kin, d_k, 1)
            for i in range(2):
                rows = slice(i * 128, (i + 1) * 128)
                nc.sync.dma_start(out=wmt[i], in_=d_wmt[rows, :].bitcast(F32R))
                nc.sync.dma_start(out=bm[i], in_=d_bm[rows, :])

            # vt ones columns; zero scratch for K-tile padding
            nc.gpsimd.memset(vt[:, :, :, HD], 1.0)
            nc.gpsimd.memset(zscr[:], 0.0)

            def zero_k_half(h):
                hp = h % 2
                nc.vector.tensor_copy(
                    out=k_sb[h][(1 - hp) * 64:(2 - hp) * 64, :], in_=zscr[:])

            zero_k_half(0)
            zero_k_half(1)

            # ---- projection helpers (512-wide rounds, shared psum pool) ----
            def q_round(mh, nt, eng):
                # writes q_sb[mh][:, nt*512:(nt+1)*512]
                ps = mix.tile([128, NQ], F32, tag="mix", name="mixq")
                cols = slice(nt * NQ, (nt + 1) * NQ)
                for ih in range(2):
                    nc.tensor.matmul(
                        ps[:], wqt[ih][:, mh * 128:(mh + 1) * 128],
                        qin[ih][:, cols], start=(ih == 0), stop=(ih == 1),
                        skip_group_check=True)
                if eng == "v":
                    nc.vector.tensor_scalar_add(
                        out=q_sb[mh][:, cols], in0=ps[:], scalar1=bq[mh])
                else:
                    nc.scalar.activation(
                        out=q_sb[mh][:, cols], in_=ps[:], func=Ident,
                        bias=bq[mh], scale=1.0)

            def k_round(mh, nt, use_act):
                # rows 0:64 -> head 2mh tile, rows 64:128 -> head 2mh+1
                ps = mix.tile([128, NQ], F32, tag="mix", name="mixk")
                cols = slice(nt * NQ, (nt + 1) * NQ)
                for ih in range(2):
                    nc.tensor.matmul(
                        ps[:], wkt[ih][:, mh * 128:(mh + 1) * 128],
                        kin[ih][:, cols], start=(ih == 0), stop=(ih == 1),
                        skip_group_check=True)
                nc.vector.tensor_scalar_add(
                    out=k_sb[2 * mh][0:64, cols], in0=ps[0:64, :],
                    scalar1=bk[mh][0:64, :])
                if use_act:
                    nc.scalar.activation(
                        out=k_sb[2 * mh + 1][64:128, cols], in_=ps[64:128, :],
                        func=Ident, bias=bk[mh][64:128, :], scale=1.0)
                else:
                    nc.vector.tensor_scalar_add(
                        out=k_sb[2 * mh + 1][64:128, cols], in0=ps[64:128, :],
                        scalar1=bk[mh][64:128, :])

            def vt_round(c, use_act):
                ps = mix.tile([128, D], F32, tag="mix", name="mixv")
                for ih in range(2):
                    nc.tensor.matmul(
                        ps[:], vin[ih][:, c * 128:(c + 1) * 128], wvt[ih][:],
                        start=(ih == 0), stop=(ih == 1), skip_group_check=True)
                view = ps[:].rearrange("p (h d) -> p h d", h=H)
                if use_act:
                    nc.scalar.copy(out=vt[:, c, :, 0:HD], in_=view)
                else:
                    nc.vector.tensor_copy(out=vt[:, c, :, 0:HD], in_=view)

            # ---- upfront: just enough for the first units ----
            for nt in range(2):
                q_round(0, nt, "v")
                k_round(0, nt, True)
            for c in range(NCHUNKS):
                vt_round(c, c % 2 == 0)

            # ---- late projections, inserted into the unit stream ----
            inserts = {
                1: [lambda: k_round(0, 2, False)],
                3: [lambda: k_round(0, 3, False)],
                5: [lambda: q_round(0, 2, "v")],
                7: [lambda: q_round(0, 3, "v")],
                10: [lambda: zero_k_half(2)],
                12: [lambda: zero_k_half(3)],
                16: [lambda: k_round(1, 0, False)],
                18: [lambda: k_round(1, 1, False)],
                20: [lambda: q_round(1, 0, "v")],
                22: [lambda: q_round(1, 1, "v")],
                24: [lambda: k_round(1, 2, False)],
                26: [lambda: k_round(1, 3, False)],
                28: [lambda: q_round(1, 2, "v")],
                30: [lambda: q_round(1, 3, "v")],
            }

            # ---- attention units ----
            NG = NUNITS * NCHUNKS  # 256 global chunks

            def emit_S(g):
                u, c = divmod(g, NCHUNKS)
                h, qj = divmod(u, N // NQ)
                th = h // 2
                t, p = divmod(g, 3)
                nc.tensor.matmul(
                    sr_ab[t % 2][:, p, :],
                    k_sb[h][:, c * 128:(c + 1) * 128],
                    q_sb[th][:, qj * NQ:(qj + 1) * NQ],
                    start=True, stop=True, skip_group_check=True,
                )

            def emit_exp(t, nch):
                nc.scalar.activation(
                    out=e_ab[t % 4][:, 0:nch, :],
                    in_=sr_ab[t % 2][:, 0:nch, :],
                    func=Exp,
                )

            xaccs = {}

            def emit_PV(g):
                u, c = divmod(g, NCHUNKS)
                h = u // (N // NQ)
                t, p = divmod(g, 3)
                if c == 0:
                    xaccs[u] = mix.tile([HD + 1, NQ], F32, tag="mix", name="xa")
                nc.tensor.matmul(
                    xaccs[u][:],
                    vt[:, c, h, :],
                    e_ab[t % 4][:, p, :],
                    start=(c == 0), stop=(c == NCHUNKS - 1),
                    skip_group_check=True,
                )

            def emit_norm(u):
                h, qj = divmod(u, N // NQ)
                th, hp = divmod(h, 2)
                xa = xaccs.pop(u)
                zrow = normp.tile([1, NQ], F32, tag="zrow", name="zrow")
                nc.vector.tensor_copy(out=zrow[:], in_=xa[HD:HD + 1, :])
                zrec = normp.tile([1, NQ], F32, tag="zrec", name="zrec")
                nc.vector.reciprocal_approx_fast(out=zrec[:], in_=zrow[:])
                zb = normp.tile([64, NQ], F32, tag="zb", name="zb")
                nc.gpsimd.partition_broadcast(zb[:], zrec[:])
                nc.vector.tensor_tensor(
                    out=x_sb[th][hp * 64:(hp + 1) * 64, qj * NQ:(qj + 1) * NQ],
                    in0=xa[0:HD, :],
                    in1=zb[:],
                    op=mybir.AluOpType.mult,
                )

            def emit_pv_triad(chunks):
                for g in chunks:
                    emit_PV(g)
                    if g % NCHUNKS == NCHUNKS - 1:
                        emit_norm(g // NCHUNKS)

            # emission per triad T: S(T); PV(T-2); exp(T)
            triads = [list(range(t * 3, min(t * 3 + 3, NG)))
                      for t in range((NG + 2) // 3)]
            for t, chunks in enumerate(triads):
                for fn in inserts.get(t, ()):
                    fn()
                for g in chunks:
                    emit_S(g)
                if t >= 2:
                    emit_pv_triad(triads[t - 2])
                emit_exp(t, len(chunks))
            emit_pv_triad(triads[-2])
            emit_pv_triad(triads[-1])

            # ---- output projection (512-wide rounds, shared pool) ----
            for mh in range(2):
                for nt in range(4):
                    ps = mix.tile([128, NQ], F32, tag="mix", name="mixo")
                    cols = slice(nt * NQ, (nt + 1) * NQ)
                    for ih in range(2):
                        nc.tensor.matmul(
                            ps[:], wmt[ih][:, mh * 128:(mh + 1) * 128],
                            x_sb[ih][:, cols], start=(ih == 0), stop=(ih == 1),
                            skip_group_check=True)
                    if nt % 2:
                        nc.vector.tensor_scalar_add(
                            out=o_sb[mh][:, cols], in0=ps[:], scalar1=bm[mh])
                    else:
                        nc.scalar.activation(
                            out=o_sb[mh][:, cols], in_=ps[:], func=Ident,
                            bias=bm[mh], scale=1.0)
                nc.sync.dma_start(
                    out=d_out[mh * 128:(mh + 1) * 128, :], in_=o_sb[mh][:])

    nc.finalize()
    return nc


def _get_nc():
    if "nc" not in _CACHE:
        _CACHE["nc"] = _build_nc()
    return _CACHE["nc"]


def _prep_host(Wq, bq, Wk, bk, Wv, bv, Wm, bm):
    r = np.arange(D)
    perm = (r % HD) * H + (r // HD)  # head-blocked row r -> original channel o
    s = np.float32(1.0 / np.sqrt(HD))
    f32 = np.float32
    wqt = np.ascontiguousarray((Wq[perm, :] * s).T, dtype=f32)
    bq_p = np.ascontiguousarray((bq[perm] * s)[:, None], dtype=f32)
    wkt = np.ascontiguousarray(Wk[perm, :].T, dtype=f32)
    bk_p = np.ascontiguousarray(bk[perm][:, None], dtype=f32)
    wvt = np.ascontiguousarray(Wv[perm, :].T, dtype=f32)
    wmt = np.ascontiguousarray(Wm[:, perm].T, dtype=f32)
    # V-bias folds into the output projection bias: X = X0 + bv (per row),
    # so out = Wm_hb @ X0 + (bm + Wm_hb @ bv_hb)
    bm_p = np.ascontiguousarray(
        (bm + Wm[:, perm] @ bv[perm])[:, None], dtype=f32)
    return dict(wqt=wqt, bq=bq_p, wkt=wkt, bk=bk_p, wvt=wvt,
                wmt=wmt, bm=bm_p)


def _run(inputs, trace=False):
    from concourse.bass_utils import run_bass_kernel_spmd

    query = np.ascontiguousarray(np.asarray(inputs["query"], dtype=np.float32))
    key = np.ascontiguousarray(np.asarray(inputs["key"], dtype=np.float32))
    value = np.ascontiguousarray(np.asarray(inputs["value"], dtype=np.float32))
    w = _prep_host(
        np.asarray(inputs["Wq"], np.float32), np.asarray(inputs["bq"], np.float32),
        np.asarray(inputs["Wk"], np.float32), np.asarray(inputs["bk"], np.float32),
        np.asarray(inputs["Wv"], np.float32), np.asarray(inputs["bv"], np.float32),
        np.asarray(inputs["Wm"], np.float32), np.asarray(inputs["bm"], np.float32),
    )
    in_maps = []
    for b in range(B):
        m = dict(w)
        m["query"] = np.ascontiguousarray(query[b])
        m["key"] = np.ascontiguousarray(key[b])
        m["value"] = np.ascontiguousarray(value[b])
        in_maps.append(m)
    nc = _get_nc()
    res = run_bass_kernel_spmd(nc, in_maps, core_ids=list(range(B)), trace=trace)
    out = np.stack([r["out"] for r in res.results], axis=0)
    return out, res


def kernel(**inputs):
    out, _ = _run(inputs, trace=False)
    return out


if __name__ == "__main__":
    rng = np.random.default_rng(0)
    s = 1.0 / np.sqrt(D)
    inputs = {
        "query": rng.standard_normal((B, D, N), dtype=np.float32),
        "key": rng.standard_normal((B, D, N), dtype=np.float32),
        "value": rng.standard_normal((B, D, N), dtype=np.float32),
        "Wq": rng.standard_normal((D, D), dtype=np.float32) * s,
        "bq": rng.standard_normal((D,), dtype=np.float32) * 0.01,
        "Wk": rng.standard_normal((D, D), dtype=np.float32) * s,
        "bk": rng.standard_normal((D,), dtype=np.float32) * 0.01,
        "Wv": rng.standard_normal((D, D), dtype=np.float32) * s,
        "bv": rng.standard_normal((D,), dtype=np.float32) * 0.01,
        "Wm": rng.standard_normal((D, D), dtype=np.float32) * s,
        "bm": rng.standard_normal((D,), dtype=np.float32) * 0.01,
    }
    out = kernel(**inputs)
    # numpy reference
    def proj(x, W, b):
        return np.einsum("oi,bin->bon", W, x) + b[None, :, None]
    q = proj(inputs["query"], inputs["Wq"], inputs["bq"]).reshape(B, HD, H, N)
    k = proj(inputs["key"], inputs["Wk"], inputs["bk"]).reshape(B, HD, H, N)
    v = proj(inputs["value"], inputs["Wv"], inputs["bv"]).reshape(B, HD, H, N)
    sc = np.einsum("bdhn,bdhm->bhnm", q, k) / np.sqrt(HD)
    sc = sc - sc.max(axis=-1, keepdims=True)
    p = np.exp(sc)
    p /= p.sum(axis=-1, keepdims=True)
    x = np.einsum("bhnm,bdhm->bdhn", p, v).reshape(B, D, N)
    ref = proj(x, inputs["Wm"], inputs["bm"])
    err = np.abs(out - ref)
    scale = np.abs(ref).max()
    print("abs err max:", err.max(), "scaled:", err.max() / scale)
    rel = np.linalg.norm(out - ref) / np.linalg.norm(ref)
    print("fro rel err:", rel)
